# revision 7
# baseline (speedup 1.0000x reference)
"""Trainium2 Bass kernel for nn_BaselineAttention_36172214567310 (v10).

Reference computation (einsum 'bhqk,bhkd->bhkd' sums over q, so attention
collapses to: v scaled by softmax column-sums):

    qkv = x @ w_qkv
    P = softmax(q @ k^T / 8)      per head, rows sum to 1
    colsum[k] = sum_q P[q, k]
    out = (v * colsum[:, None]) @ w_o

Sharding: 8 cores = 2 batches x 4 head-groups (4 heads each).

v10, from v9's trace findings (PE duration = streamed-cols x clock;
DoubleRow does NOT reduce per-column time; fp8 v-projection fails the
accuracy gate because v errors hit the output unaveraged):
- scores via ROW-TILED PACKED PAIRS: q/k stored fp8 with the head's
  64 d-rows duplicated into both array halves (tile_position (0,0) and
  (64,0)); the hh0 and hh1 score matmuls stream concurrently -> ~2x
  score throughput. Score cols are 43% of all PE columns.
- v projection reverted to bf16 (xT/wqkv inputs back) - fixes v9's
  4.1e-2 rel err.
- out3 (v23-head2, K=64) and out (v23-head3, K=64) merged into ONE
  K=128 phase in the tail: deletes 32 mid-kernel matmuls + 2MB DMA.
- q/k projection stays fp8-DR (half the accumulation steps).
- dup/pair and output DMAs issued from GpSimd (25ns issue vs Sync's
  665ns) on a separate DGE queue.
"""

import sys

sys.path.insert(0, "/opt/trn_rl_repo")

import numpy as np

B, S, HIDDEN = 2, 2048, 1024
NH, HD = 16, 64
HPC = 4
N_CORES = 8
P = 128
QC = S // P
NPAIR = QC // 2

_CACHE = {}


def _build():
    if "nc" in _CACHE:
        return _CACHE["nc"]

    import concourse.mybir as mybir
    import concourse.tile as tile
    from concourse import bacc

    F32 = mybir.dt.float32
    BF16 = mybir.dt.bfloat16
    FP8 = mybir.dt.float8e4
    EXP = mybir.ActivationFunctionType.Exp
    COPY = mybir.ActivationFunctionType.Copy
    ADD = mybir.AluOpType.add
    MULT = mybir.AluOpType.mult
    DR = mybir.MatmulPerfMode.DoubleRow

    nc = bacc.Bacc()
    x8_d = nc.declare_dram_parameter("x8", [P, 4, 2, S], FP8, isOutput=False)
    w8_d = nc.declare_dram_parameter("w8", [P, 4, 2, 512], FP8, isOutput=False)
    xT_d = nc.declare_dram_parameter("xT", [P, 8, S], BF16, isOutput=False)
    wv_d = nc.declare_dram_parameter("wv", [P, 8, 256], BF16, isOutput=False)
    wo_d = nc.declare_dram_parameter("wo", [P, 2, HIDDEN], BF16,
                                     isOutput=False)
    out2_d = nc.declare_dram_parameter("out2", [S, HIDDEN], BF16,
                                       isOutput=True)
    out3_d = nc.declare_dram_parameter("out3", [S, HIDDEN], BF16,
                                       isOutput=True)

    with tile.TileContext(nc) as tc:
        with tc.tile_pool(name="persist", bufs=1) as sb, \
             tc.tile_pool(name="small", bufs=1) as sm, \
             tc.tile_pool(name="stag", bufs=2) as stg, \
             tc.tile_pool(name="rsp", bufs=8) as rsp, \
             tc.tile_pool(name="outp", bufs=3) as outp, \
             tc.tile_pool(name="ps_s", bufs=2, space="PSUM") as ps_s_pool, \
             tc.tile_pool(name="ps_c", bufs=1, space="PSUM") as ps_c_pool, \
             tc.tile_pool(name="ps_f", bufs=1, space="PSUM") as ps_f_pool:

            x8t = sb.tile([P, 4, 2, S], FP8, name="x8t")
            w8t = sb.tile([P, 4, 2, 512], FP8, name="w8t")
            xt = sb.tile([P, 8, S], BF16, name="xt")
            wv_t = sb.tile([P, 8, 256], BF16, name="wvt")
            wo_t = sb.tile([P, 2, HIDDEN], BF16, name="wot")
            vt01 = sb.tile([P, S], BF16, name="vt01")
            vt23 = sb.tile([P, S], BF16, name="vt23")
            # per-head fp8 q/k with d-rows duplicated to both array halves
            qd = [sb.tile([P, S], FP8, name=f"qd{j}") for j in range(4)]
            kd = [sb.tile([P, S], FP8, name=f"kd{j}") for j in range(4)]
            e8 = [sb.tile([P, QC, S], FP8, name=f"e8_{i}") for i in range(2)]
            wr8 = [sb.tile([P, QC, P], FP8, name=f"wr8_{i}") for i in range(2)]
            dum = sm.tile([P, 1], F32, name="dum")
            dum2 = sm.tile([P, 1], F32, name="dum2")
            wsrc = sm.tile([P, 512], BF16, name="wsrc")

            # exp table preload + PE warmup (no DMA dependency)
            nc.vector.memset(dum, 0.0)
            nc.scalar.activation(dum2, dum, EXP)
            nc.vector.memset(wsrc, 0.0)
            wps = ps_c_pool.tile([P, 1024], F32, name="psc")
            for i in range(20):
                nc.tensor.matmul(wps[:, 0:512], wsrc[:, 0:128], wsrc,
                                 start=True, stop=True)

            # input DMA: lead-critical first, minimal issue count
            nc.sync.dma_start(out=w8t, in_=w8_d[0:P, :, :, :])
            nc.sync.dma_start(out=x8t[:, :, :, 0:1024],
                              in_=x8_d[0:P, :, :, 0:1024])
            nc.sync.dma_start(out=x8t[:, :, :, 1024:2048],
                              in_=x8_d[0:P, :, :, 1024:2048])
            nc.sync.dma_start(out=wo_t, in_=wo_d[0:P, :, :])
            nc.sync.dma_start(out=wv_t, in_=wv_d[0:P, :, :])
            nc.sync.dma_start(out=xt[:, :, 0:1024], in_=xT_d[0:P, :, 0:1024])
            nc.sync.dma_start(out=xt[:, :, 1024:2048],
                              in_=xT_d[0:P, :, 1024:2048])

            def dup_dma(stag, mb, c0, cn):
                # stag rows 0:64 = head-even d, 64:128 = head-odd d ->
                # duplicate each head's 64 rows into both halves of its
                # qd/kd tile (mb: 0=q01 1=k01 2=q23 3=k23)
                dst = qd if mb % 2 == 0 else kd
                pair = (mb // 2) * 2
                for h in range(2):
                    t = dst[pair + h]
                    nc.gpsimd.dma_start(out=t[0:64, c0:c0 + cn],
                                        in_=stag[64 * h:64 * h + 64, 0:cn])
                    nc.gpsimd.dma_start(out=t[64:128, c0:c0 + cn],
                                        in_=stag[64 * h:64 * h + 64, 0:cn])

            # ---------------- global filler queue ----------------
            # "qk8" item: one fp8-DR 512-col MM of the q/k projection
            #   (mb: 0=q01 1=k01 2=q23 3=k23; hh = S-half; 4 dk accum
            #   steps); last step of a half: cast to fp8 staging (DVE)
            #   and dup-DMA into qd/kd.
            # "p1" item: one bf16 512-col MM of the v projection
            #   (mc: 0=v01 1=v23; 8 kc accum steps) -> copy to vt.
            # "p4" item: out2 = v01 x wo0 (K=128) projection MM.
            queue = []
            for dk in range(4):                      # q01 S-half1, deadline gc8
                for n in range(2):
                    queue.append(("qk8", 0, 0, 1, dk, n))
            for hh in range(2):                      # v01 (deadlines gc15/23)
                for kc in range(8):
                    for n in range(2):
                        queue.append(("p1", 0, 0, hh, kc, n))
            for mb in (2, 3):                        # q23, k23 (deadline gc30)
                for hh in range(2):
                    for dk in range(4):
                        for n in range(2):
                            queue.append(("qk8", 0, mb, hh, dk, n))
            for hh in range(2):                      # v23 (deadlines gc47/55)
                for kc in range(8):
                    for n in range(2):
                        queue.append(("p1", 0, 1, hh, kc, n))
            for sc in range(8):                      # A sc0-7: vscale(1,0)@31
                for n in range(2):
                    queue.append(("p4", 32, "A", sc, n))
            for sc in range(8, QC):                  # A sc8-15: vscale(1,1)@39
                for n in range(2):
                    queue.append(("p4", 40, "A", sc, n))

            f_state = {"i": 0, "ps": None, "ps4": None}

            def emit_item(it):
                if it[0] == "qk8":
                    _, _, mb, hh, dk, n = it
                    if dk == 0 and n == 0:
                        f_state["ps"] = ps_f_pool.tile([P, 1024], F32,
                                                       name="psf")
                    ps = f_state["ps"]
                    c0 = hh * 1024 + n * 512
                    nc.tensor.matmul(
                        ps[:, n * 512:(n + 1) * 512],
                        w8t[:, dk, :, mb * P:(mb + 1) * P],
                        x8t[:, dk, :, c0:c0 + 512],
                        perf_mode=DR,
                        start=(dk == 0), stop=(dk == 3))
                    if dk == 3 and n == 1:
                        stag = stg.tile([P, 1024], FP8, name="stag")
                        nc.vector.tensor_copy(out=stag, in_=ps)
                        dup_dma(stag, mb, hh * 1024, 1024)
                    return
                if it[0] == "p1":
                    _, _, mc, hh, kc, n = it
                    if kc == 0 and n == 0:
                        f_state["ps"] = ps_f_pool.tile([P, 1024], F32,
                                                       name="psf")
                    ps = f_state["ps"]
                    c0 = hh * 1024 + n * 512
                    nc.tensor.matmul(
                        ps[:, n * 512:(n + 1) * 512],
                        wv_t[:, kc, mc * P:(mc + 1) * P],
                        xt[:, kc, c0:c0 + 512],
                        start=(kc == 0), stop=(kc == 7))
                    if kc == 7 and n == 1:
                        vt = vt01 if mc == 0 else vt23
                        nc.vector.tensor_copy(
                            out=vt[:, hh * 1024:(hh + 1) * 1024], in_=ps)
                else:
                    _, _, which, sc, n = it
                    if n == 0:
                        f_state["ps4"] = ps_f_pool.tile([P, 1024], F32,
                                                        name="psf")
                    ps4 = f_state["ps4"]
                    nc.tensor.matmul(
                        ps4[:, n * 512:(n + 1) * 512],
                        vt01[:, sc * P:(sc + 1) * P],
                        wo_t[:, 0, n * 512:(n + 1) * 512],
                        start=True, stop=True)
                    if n == 1:
                        o_sb = outp.tile([P, HIDDEN], BF16, name="osb")
                        nc.vector.tensor_copy(out=o_sb, in_=ps4)
                        nc.gpsimd.dma_start(
                            out=out2_d[sc * P:(sc + 1) * P, :], in_=o_sb)

            def pump(gc, budget):
                while budget > 0 and f_state["i"] < len(queue):
                    it = queue[f_state["i"]]
                    if it[1] > gc:
                        return
                    f_state["i"] += 1
                    emit_item(it)
                    budget -= 1

            def target(gc):
                if gc < 8:
                    return 4 * (gc + 1)
                if gc < 32:
                    return 32 + 3 * (gc - 7)
                return min(len(queue), 104 + 3 * (gc - 31))

            # ---------------- lead: q01 S-half0, k01 both halves ----------
            def emit_lead(mb, hh, cast_eng):
                ps = ps_s_pool.tile([P, 1024], F32, name="pss")
                for dk in range(4):
                    for n in range(2):
                        c0 = hh * 1024 + n * 512
                        nc.tensor.matmul(
                            ps[:, n * 512:(n + 1) * 512],
                            w8t[:, dk, :, mb * P:(mb + 1) * P],
                            x8t[:, dk, :, c0:c0 + 512],
                            perf_mode=DR,
                            start=(dk == 0), stop=(dk == 3))
                stag = stg.tile([P, 1024], FP8, name="stag")
                if cast_eng == "act":
                    nc.scalar.activation(stag, ps, COPY)
                else:
                    nc.vector.tensor_copy(out=stag, in_=ps)
                dup_dma(stag, mb, hh * 1024, 1024)

            emit_lead(0, 0, "act")
            emit_lead(1, 0, "vec")
            emit_lead(1, 1, "vec")

            # ---------------- head loop ----------------
            def matvec(j, half, pr, psc, first, last):
                eb, wb = e8[j % 2], wr8[j % 2]
                for n in range(2):
                    c0 = half * 1024 + n * 512
                    nc.tensor.matmul(
                        psc[:, n * 512:(n + 1) * 512],
                        wb[:, 2 * pr:2 * pr + 2, :],
                        eb[:, 2 * pr:2 * pr + 2, c0:c0 + 512],
                        perf_mode=DR,
                        start=first, stop=last)

            def vscale(j, half, psc):
                vt = vt01 if j < 2 else vt23
                bp = (j % 2) * 64
                c0 = half * 1024
                nc.vector.tensor_tensor(
                    vt[bp:bp + 64, c0:c0 + 1024], vt[bp:bp + 64, c0:c0 + 1024],
                    psc[bp:bp + 64, :], MULT)

            for j in range(HPC):
                qdj, kdj = qd[j], kd[j]
                eb, wb = e8[j % 2], wr8[j % 2]

                for qc in range(QC):
                    gc = j * QC + qc
                    # packed score pair: hh0 in array rows 0-63,
                    # hh1 in rows 64-127, streaming concurrently
                    ps_h = [ps_s_pool.tile([P, 1024], F32, name="pss")
                            for _ in range(2)]
                    for n in range(2):
                        nc.tensor.matmul(
                            ps_h[0][:, n * 512:(n + 1) * 512],
                            qdj[0:64, qc * P:(qc + 1) * P],
                            kdj[0:64, n * 512:(n + 1) * 512],
                            tile_position=(0, 0),
                            start=True, stop=True)
                        nc.tensor.matmul(
                            ps_h[1][:, n * 512:(n + 1) * 512],
                            qdj[64:128, qc * P:(qc + 1) * P],
                            kdj[64:128, 1024 + n * 512:1024 + (n + 1) * 512],
                            tile_position=(64, 0),
                            start=True, stop=True)
                    r_h = [None, None]
                    for hh in range(2):
                        r = rsp.tile([P, 1], F32, name=f"r{hh}")
                        nc.scalar.activation(
                            eb[:, qc, hh * 1024:(hh + 1) * 1024],
                            ps_h[hh], EXP, scale=0.125, accum_out=r)
                        r_h[hh] = r
                        if hh == 0:
                            due = target(gc) - f_state["i"]
                            pump(gc, max(0, min(3, (due + 1) // 2)))
                    rs = rsp.tile([P, 1], F32, name="rs")
                    nc.vector.tensor_tensor(rs, r_h[0], r_h[1], ADD)
                    rinv = rsp.tile([P, 1], F32, name="rinv")
                    nc.vector.reciprocal(rinv, rs)
                    nc.vector.tensor_scalar(wb[:, qc, :],
                                            rinv.to_broadcast([P, P]),
                                            1024.0, None, MULT)

                    # colsum matvec scheduling
                    if j < 3:
                        if j > 0 and qc < NPAIR:
                            if qc == 0:
                                f_state["psc"] = ps_c_pool.tile(
                                    [P, 1024], F32, name="psc")
                            matvec(j - 1, 1, qc, f_state["psc"],
                                   qc == 0, qc == NPAIR - 1)
                            if qc == NPAIR - 1:
                                vscale(j - 1, 1, f_state["psc"])
                        elif qc >= NPAIR:
                            pr = qc - NPAIR
                            if pr == 0:
                                f_state["psc"] = ps_c_pool.tile(
                                    [P, 1024], F32, name="psc")
                            matvec(j, 0, pr, f_state["psc"],
                                   pr == 0, pr == NPAIR - 1)
                            if pr == NPAIR - 1:
                                vscale(j, 0, f_state["psc"])
                    else:
                        if qc < NPAIR:
                            if qc == 0:
                                f_state["psc"] = ps_c_pool.tile(
                                    [P, 1024], F32, name="psc")
                            matvec(2, 1, qc, f_state["psc"],
                                   qc == 0, qc == NPAIR - 1)
                            if qc == NPAIR - 1:
                                vscale(2, 1, f_state["psc"])
                        elif qc >= 12:
                            # head 3: both colsum halves, 2 steps each/slot
                            if qc == 12:
                                f_state["psc"] = ps_c_pool.tile(
                                    [P, 1024], F32, name="psc")
                                f_state["psc2"] = ps_f_pool.tile(
                                    [P, 1024], F32, name="psf")
                            for t in range(2):
                                pr = (qc - 12) * 2 + t
                                matvec(3, 0, pr, f_state["psc"],
                                       pr == 0, pr == NPAIR - 1)
                                matvec(3, 1, pr, f_state["psc2"],
                                       pr == 0, pr == NPAIR - 1)
                    pump(gc, max(0, min(5, target(gc) - f_state["i"])))

            # ------- tail: vscale head3 + merged out3 (v23 full, K=128) ----
            vscale(3, 0, f_state["psc"])
            vscale(3, 1, f_state["psc2"])
            pump(63, len(queue))
            for sc in range(QC):
                ps_o = ps_s_pool.tile([P, 1024], F32, name="pss")
                for n in range(2):
                    nc.tensor.matmul(
                        ps_o[:, n * 512:(n + 1) * 512],
                        vt23[:, sc * P:(sc + 1) * P],
                        wo_t[:, 1, n * 512:(n + 1) * 512],
                        start=True, stop=True)
                o_sb = outp.tile([P, HIDDEN], BF16, name="osb")
                if sc % 2 == 0:
                    nc.scalar.activation(o_sb, ps_o, COPY)
                else:
                    nc.vector.tensor_copy(out=o_sb, in_=ps_o)
                nc.gpsimd.dma_start(out=out3_d[sc * P:(sc + 1) * P, :],
                                    in_=o_sb)

    nc.compile()
    _CACHE["nc"] = nc
    return nc


def kernel(x: np.ndarray, w_qkv: np.ndarray, w_o: np.ndarray) -> np.ndarray:
    import ml_dtypes
    from concourse.bass_utils import run_bass_kernel_spmd

    nc = _build()

    def pack4d(a):
        # [1024, C] -> [128, 4, 2, C]: row dk*256 + i*128 + p -> [p, dk, i]
        cc = a.shape[1]
        return np.ascontiguousarray(
            a.reshape(4, 2, 128, cc).transpose(2, 0, 1, 3))

    def pack8(a):
        # [1024, C] -> [128, 8, C]: row kc*128 + p -> [p, kc]
        cc = a.shape[1]
        return np.ascontiguousarray(a.reshape(8, 128, cc).transpose(1, 0, 2))

    def to_fp8(a):
        return np.clip(a, -240.0, 240.0).astype(ml_dtypes.float8_e4m3)

    xTs = [np.ascontiguousarray(x[b].T) for b in range(B)]
    x8 = [to_fp8(pack4d(xTs[b])) for b in range(B)]
    xbf = [pack8(xTs[b]).astype(ml_dtypes.bfloat16) for b in range(B)]
    in_maps = []
    for c in range(N_CORES):
        b, g = divmod(c, HPC)
        base = 256 * g
        q01 = w_qkv[:, base:base + 128]
        q23 = w_qkv[:, base + 128:base + 256]
        k01 = w_qkv[:, 1024 + base:1024 + base + 128]
        k23 = w_qkv[:, 1024 + base + 128:1024 + base + 256]
        v01 = w_qkv[:, 2048 + base:2048 + base + 128]
        v23 = w_qkv[:, 2048 + base + 128:2048 + base + 256]
        wqk = np.concatenate([q01, k01, q23, k23], axis=1)
        wv = np.concatenate([v01, v23], axis=1)
        wo_slice = (w_o[base:base + 256, :] * (1.0 / 1024.0)).reshape(
            2, 128, HIDDEN).transpose(1, 0, 2)
        in_maps.append({
            "x8": x8[b],
            "w8": to_fp8(pack4d(wqk)),
            "xT": xbf[b],
            "wv": pack8(wv).astype(ml_dtypes.bfloat16),
            "wo": np.ascontiguousarray(wo_slice).astype(ml_dtypes.bfloat16),
        })

    res = run_bass_kernel_spmd(nc, in_maps, list(range(N_CORES)),
                               **_CACHE.get("run_kwargs", {}))
    _CACHE["last_result"] = res

    out = np.zeros((B, S, HIDDEN), np.float32)
    for c in range(N_CORES):
        r = res.results[c]
        out[c // HPC] += (r["out2"].astype(np.float32)
                          + r["out3"].astype(np.float32))
    return out


# revision 8
# speedup vs baseline: 1.0036x; 1.0036x over previous
"""Trainium2 Bass kernel for nn_BaselineAttention_36172214567310 (v11).

Reference computation (einsum 'bhqk,bhkd->bhkd' sums over q, so attention
collapses to: v scaled by softmax column-sums):

    qkv = x @ w_qkv
    P = softmax(q @ k^T / 8)      per head, rows sum to 1
    colsum[k] = sum_q P[q, k]
    out = (v * colsum[:, None]) @ w_o

Sharding: 8 cores = 2 batches x 4 head-groups (4 heads each).

v11 = v10 with the lead and exp-pipeline stalls fixed (v10 trace: first
exp at 30.6us from serialized strided DMA issues; exps waited ~550ns on
score matmuls queued behind filler):
- x8/xT in S-half-major layout -> each input DMA is contiguous per
  partition (issue cost tracks descriptor count).
- kd stored RESHAPED [k-half0 rows 0-63 | k-half1 rows 64-127] (no
  duplication): 2 DMAs per projection half instead of 4.
- lead dup-DMA issues split across the scalar DGE (q01, before its
  exps) and gpsimd DGE (k01, q01-Sh1); q01-Sh1 moved into the lead.
- next slot's hh0 score pair is emitted at the END of the previous
  slot, so it streams during exp(t, hh1) instead of queueing behind
  matvec/filler matmuls.
- scores stay fp8 row-tile packed; q/k projection fp8-DR; v bf16;
  out3 = (v23 full, K=128) in the tail.
"""

import sys

sys.path.insert(0, "/opt/trn_rl_repo")

import numpy as np

B, S, HIDDEN = 2, 2048, 1024
NH, HD = 16, 64
HPC = 4
N_CORES = 8
P = 128
QC = S // P
NPAIR = QC // 2

_CACHE = {}


def _build():
    if "nc" in _CACHE:
        return _CACHE["nc"]

    import concourse.mybir as mybir
    import concourse.tile as tile
    from concourse import bacc

    F32 = mybir.dt.float32
    BF16 = mybir.dt.bfloat16
    FP8 = mybir.dt.float8e4
    EXP = mybir.ActivationFunctionType.Exp
    COPY = mybir.ActivationFunctionType.Copy
    ADD = mybir.AluOpType.add
    MULT = mybir.AluOpType.mult
    DR = mybir.MatmulPerfMode.DoubleRow

    nc = bacc.Bacc()
    x8_d = nc.declare_dram_parameter("x8", [P, 2, 4, 2, 1024], FP8,
                                     isOutput=False)
    w8_d = nc.declare_dram_parameter("w8", [P, 4, 2, 512], FP8, isOutput=False)
    xT_d = nc.declare_dram_parameter("xT", [P, 2, 8, 1024], BF16,
                                     isOutput=False)
    wv_d = nc.declare_dram_parameter("wv", [P, 8, 256], BF16, isOutput=False)
    wo_d = nc.declare_dram_parameter("wo", [P, 2, HIDDEN], BF16,
                                     isOutput=False)
    out2_d = nc.declare_dram_parameter("out2", [S, HIDDEN], BF16,
                                       isOutput=True)
    out3_d = nc.declare_dram_parameter("out3", [S, HIDDEN], BF16,
                                       isOutput=True)

    with tile.TileContext(nc) as tc:
        with tc.tile_pool(name="persist", bufs=1) as sb, \
             tc.tile_pool(name="small", bufs=1) as sm, \
             tc.tile_pool(name="stag", bufs=2) as stg, \
             tc.tile_pool(name="rsp", bufs=8) as rsp, \
             tc.tile_pool(name="outp", bufs=3) as outp, \
             tc.tile_pool(name="ps_s", bufs=2, space="PSUM") as ps_s_pool, \
             tc.tile_pool(name="ps_c", bufs=1, space="PSUM") as ps_c_pool, \
             tc.tile_pool(name="ps_f", bufs=1, space="PSUM") as ps_f_pool:

            x8t = sb.tile([P, 2, 4, 2, 1024], FP8, name="x8t")
            w8t = sb.tile([P, 4, 2, 512], FP8, name="w8t")
            xt = sb.tile([P, 2, 8, 1024], BF16, name="xt")
            wv_t = sb.tile([P, 8, 256], BF16, name="wvt")
            wo_t = sb.tile([P, 2, HIDDEN], BF16, name="wot")
            vt01 = sb.tile([P, S], BF16, name="vt01")
            vt23 = sb.tile([P, S], BF16, name="vt23")
            # qd: per-head fp8 q, d-rows duplicated into both array halves.
            # kd: per-head fp8 k, rows 0-63 = k-half0, rows 64-127 = k-half1.
            qd = [sb.tile([P, S], FP8, name=f"qd{j}") for j in range(4)]
            kd = [sb.tile([P, 1024], FP8, name=f"kd{j}") for j in range(4)]
            e8 = [sb.tile([P, QC, S], FP8, name=f"e8_{i}") for i in range(2)]
            wr8 = [sb.tile([P, QC, P], FP8, name=f"wr8_{i}") for i in range(2)]
            dum = sm.tile([P, 1], F32, name="dum")
            dum2 = sm.tile([P, 1], F32, name="dum2")
            wsrc = sm.tile([P, 512], BF16, name="wsrc")

            # exp table preload + PE warmup (no DMA dependency)
            nc.vector.memset(dum, 0.0)
            nc.scalar.activation(dum2, dum, EXP)
            nc.vector.memset(wsrc, 0.0)
            wps = ps_c_pool.tile([P, 1024], F32, name="psc")
            for i in range(20):
                nc.tensor.matmul(wps[:, 0:512], wsrc[:, 0:128], wsrc,
                                 start=True, stop=True)

            # input DMA: lead-critical first; all contiguous per partition
            nc.sync.dma_start(out=w8t, in_=w8_d[0:P, :, :, :])
            nc.sync.dma_start(out=x8t[:, 0], in_=x8_d[0:P, 0])
            nc.sync.dma_start(out=x8t[:, 1], in_=x8_d[0:P, 1])
            nc.sync.dma_start(out=wv_t, in_=wv_d[0:P, :, :])
            nc.sync.dma_start(out=wo_t, in_=wo_d[0:P, :, :])
            nc.sync.dma_start(out=xt[:, 0], in_=xT_d[0:P, 0])
            nc.sync.dma_start(out=xt[:, 1], in_=xT_d[0:P, 1])

            def dup_q(stag, pair, hh, eng):
                # stag rows 0:64/64:128 = head-even/odd q d-rows for S-half hh
                for h in range(2):
                    t = qd[pair + h]
                    src = stag[64 * h:64 * h + 64, :]
                    eng.dma_start(out=t[0:64, hh * 1024:(hh + 1) * 1024],
                                  in_=src)
                    eng.dma_start(out=t[64:128, hh * 1024:(hh + 1) * 1024],
                                  in_=src)

            def dup_k(stag, pair, hh, eng):
                # k-half hh -> rows 64*hh..64*hh+63 of each head's kd
                for h in range(2):
                    eng.dma_start(
                        out=kd[pair + h][64 * hh:64 * hh + 64, :],
                        in_=stag[64 * h:64 * h + 64, :])

            # ---------------- global filler queue ----------------
            # "qk8": one fp8-DR 512-col MM of the q/k projection (mb: 2=q23
            #   3=k23; hh = S-half for q, k-half for k; 4 dk accum steps);
            #   last step: cast to fp8 staging (DVE), dup/reshape-DMA.
            # "p1": one bf16 512-col MM of the v projection (mc 0=v01 1=v23).
            # "p4": out2 = v01 x wo0 (K=128) projection MM.
            queue = []
            for hh in range(2):                      # v01 (deadlines gc15/23)
                for kc in range(8):
                    for n in range(2):
                        queue.append(("p1", 0, 0, hh, kc, n))
            for mb in (2, 3):                        # q23, k23 (deadline gc30)
                for hh in range(2):
                    for dk in range(4):
                        for n in range(2):
                            queue.append(("qk8", 0, mb, hh, dk, n))
            for hh in range(2):                      # v23 (deadlines gc47/55)
                for kc in range(8):
                    for n in range(2):
                        queue.append(("p1", 0, 1, hh, kc, n))
            for sc in range(8):                      # A sc0-7: vscale(1,0)@31
                for n in range(2):
                    queue.append(("p4", 32, "A", sc, n))
            for sc in range(8, QC):                  # A sc8-15: vscale(1,1)@39
                for n in range(2):
                    queue.append(("p4", 40, "A", sc, n))

            f_state = {"i": 0, "ps": None, "ps4": None}

            def emit_item(it):
                if it[0] == "qk8":
                    _, _, mb, hh, dk, n = it
                    if dk == 0 and n == 0:
                        f_state["ps"] = ps_f_pool.tile([P, 1024], F32,
                                                       name="psf")
                    ps = f_state["ps"]
                    nc.tensor.matmul(
                        ps[:, n * 512:(n + 1) * 512],
                        w8t[:, dk, :, mb * P:(mb + 1) * P],
                        x8t[:, hh, dk, :, n * 512:(n + 1) * 512],
                        perf_mode=DR,
                        start=(dk == 0), stop=(dk == 3))
                    if dk == 3 and n == 1:
                        stag = stg.tile([P, 1024], FP8, name="stag")
                        nc.vector.tensor_copy(out=stag, in_=ps)
                        if mb == 2:
                            dup_q(stag, 2, hh, nc.gpsimd)
                        else:
                            dup_k(stag, 2, hh, nc.gpsimd)
                    return
                if it[0] == "p1":
                    _, _, mc, hh, kc, n = it
                    if kc == 0 and n == 0:
                        f_state["ps"] = ps_f_pool.tile([P, 1024], F32,
                                                       name="psf")
                    ps = f_state["ps"]
                    nc.tensor.matmul(
                        ps[:, n * 512:(n + 1) * 512],
                        wv_t[:, kc, mc * P:(mc + 1) * P],
                        xt[:, hh, kc, n * 512:(n + 1) * 512],
                        start=(kc == 0), stop=(kc == 7))
                    if kc == 7 and n == 1:
                        vt = vt01 if mc == 0 else vt23
                        nc.vector.tensor_copy(
                            out=vt[:, hh * 1024:(hh + 1) * 1024], in_=ps)
                else:
                    _, _, which, sc, n = it
                    if n == 0:
                        f_state["ps4"] = ps_f_pool.tile([P, 1024], F32,
                                                        name="psf")
                    ps4 = f_state["ps4"]
                    nc.tensor.matmul(
                        ps4[:, n * 512:(n + 1) * 512],
                        vt01[:, sc * P:(sc + 1) * P],
                        wo_t[:, 0, n * 512:(n + 1) * 512],
                        start=True, stop=True)
                    if n == 1:
                        o_sb = outp.tile([P, HIDDEN], BF16, name="osb")
                        nc.vector.tensor_copy(out=o_sb, in_=ps4)
                        nc.gpsimd.dma_start(
                            out=out2_d[sc * P:(sc + 1) * P, :], in_=o_sb)

            def pump(gc, budget):
                while budget > 0 and f_state["i"] < len(queue):
                    it = queue[f_state["i"]]
                    if it[1] > gc:
                        return
                    f_state["i"] += 1
                    emit_item(it)
                    budget -= 1

            def target(gc):
                if gc < 8:
                    return 4 * (gc + 1)
                if gc < 32:
                    return 32 + 3 * (gc - 7)
                return min(len(queue), 104 + 3 * (gc - 31))

            # -------- lead: q01 both S-halves, k01 both k-halves ----------
            def emit_lead(mb, hh, is_q, cast_eng, dma_eng):
                ps = ps_s_pool.tile([P, 1024], F32, name="pss")
                for dk in range(4):
                    for n in range(2):
                        nc.tensor.matmul(
                            ps[:, n * 512:(n + 1) * 512],
                            w8t[:, dk, :, mb * P:(mb + 1) * P],
                            x8t[:, hh, dk, :, n * 512:(n + 1) * 512],
                            perf_mode=DR,
                            start=(dk == 0), stop=(dk == 3))
                stag = stg.tile([P, 1024], FP8, name="stag")
                if cast_eng == "act":
                    nc.scalar.activation(stag, ps, COPY)
                else:
                    nc.vector.tensor_copy(out=stag, in_=ps)
                if is_q:
                    dup_q(stag, 0, hh, dma_eng)
                else:
                    dup_k(stag, 0, hh, dma_eng)

            emit_lead(0, 0, True, "act", nc.scalar)    # q01 S-half0
            emit_lead(1, 0, False, "vec", nc.gpsimd)   # k01 k-half0
            emit_lead(1, 1, False, "vec", nc.gpsimd)   # k01 k-half1
            emit_lead(0, 1, True, "vec", nc.gpsimd)    # q01 S-half1

            # ---------------- head loop ----------------
            def score_pair(j, qc, ps_tile, hh):
                qdj, kdj = qd[j], kd[j]
                rb = 64 * hh
                for n in range(2):
                    nc.tensor.matmul(
                        ps_tile[:, n * 512:(n + 1) * 512],
                        qdj[rb:rb + 64, qc * P:(qc + 1) * P],
                        kdj[rb:rb + 64, n * 512:(n + 1) * 512],
                        tile_position=(rb, 0),
                        start=True, stop=True)

            def matvec(j, half, pr, psc, first, last):
                eb, wb = e8[j % 2], wr8[j % 2]
                for n in range(2):
                    c0 = half * 1024 + n * 512
                    nc.tensor.matmul(
                        psc[:, n * 512:(n + 1) * 512],
                        wb[:, 2 * pr:2 * pr + 2, :],
                        eb[:, 2 * pr:2 * pr + 2, c0:c0 + 512],
                        perf_mode=DR,
                        start=first, stop=last)

            def vscale(j, half, psc):
                vt = vt01 if j < 2 else vt23
                bp = (j % 2) * 64
                c0 = half * 1024
                nc.vector.tensor_tensor(
                    vt[bp:bp + 64, c0:c0 + 1024], vt[bp:bp + 64, c0:c0 + 1024],
                    psc[bp:bp + 64, :], MULT)

            # first slot's hh0 scores, then steady state: emit hh0 of the
            # NEXT slot at the end of each slot.
            ps_next = ps_s_pool.tile([P, 1024], F32, name="pss")
            score_pair(0, 0, ps_next, 0)

            for j in range(HPC):
                eb, wb = e8[j % 2], wr8[j % 2]

                for qc in range(QC):
                    gc = j * QC + qc
                    ps_h0 = ps_next
                    ps_h1 = ps_s_pool.tile([P, 1024], F32, name="pss")
                    score_pair(j, qc, ps_h1, 1)
                    r_h = [None, None]
                    for hh in range(2):
                        r = rsp.tile([P, 1], F32, name=f"r{hh}")
                        nc.scalar.activation(
                            eb[:, qc, hh * 1024:(hh + 1) * 1024],
                            ps_h0 if hh == 0 else ps_h1,
                            EXP, scale=0.125, accum_out=r)
                        r_h[hh] = r
                        if hh == 0:
                            due = target(gc) - f_state["i"]
                            pump(gc, max(0, min(3, (due + 1) // 2)))
                    rs = rsp.tile([P, 1], F32, name="rs")
                    nc.vector.tensor_tensor(rs, r_h[0], r_h[1], ADD)
                    rinv = rsp.tile([P, 1], F32, name="rinv")
                    nc.vector.reciprocal(rinv, rs)
                    nc.vector.tensor_scalar(wb[:, qc, :],
                                            rinv.to_broadcast([P, P]),
                                            1024.0, None, MULT)

                    # colsum matvec scheduling
                    if j < 3:
                        if j > 0 and qc < NPAIR:
                            if qc == 0:
                                f_state["psc"] = ps_c_pool.tile(
                                    [P, 1024], F32, name="psc")
                            matvec(j - 1, 1, qc, f_state["psc"],
                                   qc == 0, qc == NPAIR - 1)
                            if qc == NPAIR - 1:
                                vscale(j - 1, 1, f_state["psc"])
                        elif qc >= NPAIR:
                            pr = qc - NPAIR
                            if pr == 0:
                                f_state["psc"] = ps_c_pool.tile(
                                    [P, 1024], F32, name="psc")
                            matvec(j, 0, pr, f_state["psc"],
                                   pr == 0, pr == NPAIR - 1)
                            if pr == NPAIR - 1:
                                vscale(j, 0, f_state["psc"])
                    else:
                        if qc < NPAIR:
                            if qc == 0:
                                f_state["psc"] = ps_c_pool.tile(
                                    [P, 1024], F32, name="psc")
                            matvec(2, 1, qc, f_state["psc"],
                                   qc == 0, qc == NPAIR - 1)
                            if qc == NPAIR - 1:
                                vscale(2, 1, f_state["psc"])
                        elif qc >= 12:
                            # head 3: both colsum halves, 2 steps each/slot
                            if qc == 12:
                                f_state["psc"] = ps_c_pool.tile(
                                    [P, 1024], F32, name="psc")
                                f_state["psc2"] = ps_f_pool.tile(
                                    [P, 1024], F32, name="psf")
                            for t in range(2):
                                pr = (qc - 12) * 2 + t
                                matvec(3, 0, pr, f_state["psc"],
                                       pr == 0, pr == NPAIR - 1)
                                matvec(3, 1, pr, f_state["psc2"],
                                       pr == 0, pr == NPAIR - 1)
                    pump(gc, max(0, min(5, target(gc) - f_state["i"])))
                    # next slot's hh0 scores (stream during exp(t, hh1))
                    if gc < HPC * QC - 1:
                        nj, nqc = (j, qc + 1) if qc + 1 < QC else (j + 1, 0)
                        ps_next = ps_s_pool.tile([P, 1024], F32, name="pss")
                        score_pair(nj, nqc, ps_next, 0)

            # ------- tail: vscale head3 + merged out3 (v23 full, K=128) ----
            vscale(3, 0, f_state["psc"])
            vscale(3, 1, f_state["psc2"])
            pump(63, len(queue))
            for sc in range(QC):
                ps_o = ps_s_pool.tile([P, 1024], F32, name="pss")
                for n in range(2):
                    nc.tensor.matmul(
                        ps_o[:, n * 512:(n + 1) * 512],
                        vt23[:, sc * P:(sc + 1) * P],
                        wo_t[:, 1, n * 512:(n + 1) * 512],
                        start=True, stop=True)
                o_sb = outp.tile([P, HIDDEN], BF16, name="osb")
                if sc % 2 == 0:
                    nc.scalar.activation(o_sb, ps_o, COPY)
                else:
                    nc.vector.tensor_copy(out=o_sb, in_=ps_o)
                nc.gpsimd.dma_start(out=out3_d[sc * P:(sc + 1) * P, :],
                                    in_=o_sb)

    nc.compile()
    _CACHE["nc"] = nc
    return nc


def kernel(x: np.ndarray, w_qkv: np.ndarray, w_o: np.ndarray) -> np.ndarray:
    import ml_dtypes
    from concourse.bass_utils import run_bass_kernel_spmd

    nc = _build()

    def pack4d(a):
        # [1024, C] -> [128, 4, 2, C]: row dk*256 + i*128 + p -> [p, dk, i]
        cc = a.shape[1]
        return np.ascontiguousarray(
            a.reshape(4, 2, 128, cc).transpose(2, 0, 1, 3))

    def to_fp8(a):
        return np.clip(a, -240.0, 240.0).astype(ml_dtypes.float8_e4m3)

    xTs = [np.ascontiguousarray(x[b].T) for b in range(B)]
    # x8: [128, 2(S-half), 4, 2, 1024]
    x8 = [to_fp8(np.ascontiguousarray(
        xTs[b].reshape(4, 2, 128, 2, 1024).transpose(2, 3, 0, 1, 4)))
        for b in range(B)]
    # xT: [128, 2(S-half), 8, 1024]
    xbf = [np.ascontiguousarray(
        xTs[b].reshape(8, 128, 2, 1024).transpose(1, 2, 0, 3)).astype(
            ml_dtypes.bfloat16) for b in range(B)]
    in_maps = []
    for c in range(N_CORES):
        b, g = divmod(c, HPC)
        base = 256 * g
        q01 = w_qkv[:, base:base + 128]
        q23 = w_qkv[:, base + 128:base + 256]
        k01 = w_qkv[:, 1024 + base:1024 + base + 128]
        k23 = w_qkv[:, 1024 + base + 128:1024 + base + 256]
        v01 = w_qkv[:, 2048 + base:2048 + base + 128]
        v23 = w_qkv[:, 2048 + base + 128:2048 + base + 256]
        wqk = np.concatenate([q01, k01, q23, k23], axis=1)
        wv = np.concatenate([v01, v23], axis=1)
        wo_slice = (w_o[base:base + 256, :] * (1.0 / 1024.0)).reshape(
            2, 128, HIDDEN).transpose(1, 0, 2)
        in_maps.append({
            "x8": x8[b],
            "w8": to_fp8(pack4d(wqk)),
            "xT": xbf[b],
            "wv": np.ascontiguousarray(
                wv.reshape(8, 128, 256).transpose(1, 0, 2)).astype(
                    ml_dtypes.bfloat16),
            "wo": np.ascontiguousarray(wo_slice).astype(ml_dtypes.bfloat16),
        })

    res = run_bass_kernel_spmd(nc, in_maps, list(range(N_CORES)),
                               **_CACHE.get("run_kwargs", {}))
    _CACHE["last_result"] = res

    out = np.zeros((B, S, HIDDEN), np.float32)
    for c in range(N_CORES):
        r = res.results[c]
        out[c // HPC] += (r["out2"].astype(np.float32)
                          + r["out3"].astype(np.float32))
    return out


# revision 9
# speedup vs baseline: 1.0039x; 1.0003x over previous
"""Trainium2 Bass kernel for nn_BaselineAttention_36172214567310 (v11).

Reference computation (einsum 'bhqk,bhkd->bhkd' sums over q, so attention
collapses to: v scaled by softmax column-sums):

    qkv = x @ w_qkv
    P = softmax(q @ k^T / 8)      per head, rows sum to 1
    colsum[k] = sum_q P[q, k]
    out = (v * colsum[:, None]) @ w_o

Sharding: 8 cores = 2 batches x 4 head-groups (4 heads each).

v11 = v10 with the lead and exp-pipeline stalls fixed (v10 trace: first
exp at 30.6us from serialized strided DMA issues; exps waited ~550ns on
score matmuls queued behind filler):
- x8/xT in S-half-major layout -> each input DMA is contiguous per
  partition (issue cost tracks descriptor count).
- kd stored RESHAPED [k-half0 rows 0-63 | k-half1 rows 64-127] (no
  duplication): 2 DMAs per projection half instead of 4.
- lead dup-DMA issues split across the scalar DGE (q01, before its
  exps) and gpsimd DGE (k01, q01-Sh1); q01-Sh1 moved into the lead.
- next slot's hh0 score pair is emitted at the END of the previous
  slot, so it streams during exp(t, hh1) instead of queueing behind
  matvec/filler matmuls.
- scores stay fp8 row-tile packed; q/k projection fp8-DR; v bf16;
  out3 = (v23 full, K=128) in the tail.
"""

import sys

sys.path.insert(0, "/opt/trn_rl_repo")

import numpy as np

B, S, HIDDEN = 2, 2048, 1024
NH, HD = 16, 64
HPC = 4
N_CORES = 8
P = 128
QC = S // P
NPAIR = QC // 2

_CACHE = {}


def _build():
    if "nc" in _CACHE:
        return _CACHE["nc"]

    import concourse.mybir as mybir
    import concourse.tile as tile
    from concourse import bacc

    F32 = mybir.dt.float32
    BF16 = mybir.dt.bfloat16
    FP8 = mybir.dt.float8e4
    EXP = mybir.ActivationFunctionType.Exp
    COPY = mybir.ActivationFunctionType.Copy
    ADD = mybir.AluOpType.add
    MULT = mybir.AluOpType.mult
    DR = mybir.MatmulPerfMode.DoubleRow

    nc = bacc.Bacc()
    x8_d = nc.declare_dram_parameter("x8", [P, 2, 4, 2, 1024], FP8,
                                     isOutput=False)
    w8_d = nc.declare_dram_parameter("w8", [P, 2, 4, 2, 256], FP8,
                                     isOutput=False)
    xT_d = nc.declare_dram_parameter("xT", [P, 2, 8, 1024], BF16,
                                     isOutput=False)
    wv_d = nc.declare_dram_parameter("wv", [P, 8, 256], BF16, isOutput=False)
    wo_d = nc.declare_dram_parameter("wo", [P, 2, HIDDEN], BF16,
                                     isOutput=False)
    out2_d = nc.declare_dram_parameter("out2", [S, HIDDEN], BF16,
                                       isOutput=True)
    out3_d = nc.declare_dram_parameter("out3", [S, HIDDEN], BF16,
                                       isOutput=True)

    with tile.TileContext(nc) as tc:
        with tc.tile_pool(name="persist", bufs=1) as sb, \
             tc.tile_pool(name="small", bufs=1) as sm, \
             tc.tile_pool(name="stag", bufs=2) as stg, \
             tc.tile_pool(name="rsp", bufs=8) as rsp, \
             tc.tile_pool(name="outp", bufs=3) as outp, \
             tc.tile_pool(name="ps_s", bufs=2, space="PSUM") as ps_s_pool, \
             tc.tile_pool(name="ps_c", bufs=1, space="PSUM") as ps_c_pool, \
             tc.tile_pool(name="ps_f", bufs=1, space="PSUM") as ps_f_pool:

            x8t = sb.tile([P, 2, 4, 2, 1024], FP8, name="x8t")
            w8t = sb.tile([P, 2, 4, 2, 256], FP8, name="w8t")
            xt = sb.tile([P, 2, 8, 1024], BF16, name="xt")
            wv_t = sb.tile([P, 8, 256], BF16, name="wvt")
            wo_t = sb.tile([P, 2, HIDDEN], BF16, name="wot")
            vt01 = sb.tile([P, S], BF16, name="vt01")
            vt23 = sb.tile([P, S], BF16, name="vt23")
            # qd: per-head fp8 q, d-rows duplicated into both array halves.
            # kd: per-head fp8 k, rows 0-63 = k-half0, rows 64-127 = k-half1.
            qd = [sb.tile([P, S], FP8, name=f"qd{j}") for j in range(4)]
            kd = [sb.tile([P, 1024], FP8, name=f"kd{j}") for j in range(4)]
            e8 = [sb.tile([P, QC, S], FP8, name=f"e8_{i}") for i in range(2)]
            wr8 = [sb.tile([P, QC, P], FP8, name=f"wr8_{i}") for i in range(2)]
            dum = sm.tile([P, 1], F32, name="dum")
            dum2 = sm.tile([P, 1], F32, name="dum2")
            wsrc = sm.tile([P, 512], BF16, name="wsrc")

            # exp table preload + PE warmup (no DMA dependency)
            nc.vector.memset(dum, 0.0)
            nc.scalar.activation(dum2, dum, EXP)
            nc.vector.memset(wsrc, 0.0)
            wps = ps_c_pool.tile([P, 1024], F32, name="psc")
            for i in range(20):
                nc.tensor.matmul(wps[:, 0:512], wsrc[:, 0:128], wsrc,
                                 start=True, stop=True)

            # input DMA: lead-critical first; all contiguous per partition
            nc.sync.dma_start(out=w8t[:, 0], in_=w8_d[0:P, 0])
            nc.sync.dma_start(out=x8t[:, 0], in_=x8_d[0:P, 0])
            nc.sync.dma_start(out=w8t[:, 1], in_=w8_d[0:P, 1])
            nc.sync.dma_start(out=x8t[:, 1], in_=x8_d[0:P, 1])
            nc.sync.dma_start(out=xt[:, 0], in_=xT_d[0:P, 0])
            nc.sync.dma_start(out=wv_t, in_=wv_d[0:P, :, :])
            nc.sync.dma_start(out=xt[:, 1], in_=xT_d[0:P, 1])
            nc.sync.dma_start(out=wo_t, in_=wo_d[0:P, :, :])

            def dup_q(stag, pair, hh, eng):
                # stag rows 0:64/64:128 = head-even/odd q d-rows for S-half hh
                for h in range(2):
                    t = qd[pair + h]
                    src = stag[64 * h:64 * h + 64, :]
                    eng.dma_start(out=t[0:64, hh * 1024:(hh + 1) * 1024],
                                  in_=src)
                    eng.dma_start(out=t[64:128, hh * 1024:(hh + 1) * 1024],
                                  in_=src)

            def dup_k(stag, pair, hh, eng):
                # k-half hh -> rows 64*hh..64*hh+63 of each head's kd
                for h in range(2):
                    eng.dma_start(
                        out=kd[pair + h][64 * hh:64 * hh + 64, :],
                        in_=stag[64 * h:64 * h + 64, :])

            # ---------------- global filler queue ----------------
            # "qk8": one fp8-DR 512-col MM of the q/k projection (mb: 2=q23
            #   3=k23; hh = S-half for q, k-half for k; 4 dk accum steps);
            #   last step: cast to fp8 staging (DVE), dup/reshape-DMA.
            # "p1": one bf16 512-col MM of the v projection (mc 0=v01 1=v23).
            # "p4": out2 = v01 x wo0 (K=128) projection MM.
            queue = []
            for hh in range(2):                      # v01 (deadlines gc15/23)
                for kc in range(8):
                    for n in range(2):
                        queue.append(("p1", 0, 0, hh, kc, n))
            for mb in (2, 3):                        # q23, k23 (deadline gc30)
                for hh in range(2):
                    for dk in range(4):
                        for n in range(2):
                            queue.append(("qk8", 0, mb, hh, dk, n))
            for hh in range(2):                      # v23 (deadlines gc47/55)
                for kc in range(8):
                    for n in range(2):
                        queue.append(("p1", 0, 1, hh, kc, n))
            for sc in range(8):                      # A sc0-7: vscale(1,0)@31
                for n in range(2):
                    queue.append(("p4", 32, "A", sc, n))
            for sc in range(8, QC):                  # A sc8-15: vscale(1,1)@39
                for n in range(2):
                    queue.append(("p4", 40, "A", sc, n))

            f_state = {"i": 0, "ps": None, "ps4": None}

            def emit_item(it):
                if it[0] == "qk8":
                    _, _, mb, hh, dk, n = it
                    if dk == 0 and n == 0:
                        f_state["ps"] = ps_f_pool.tile([P, 1024], F32,
                                                       name="psf")
                    ps = f_state["ps"]
                    nc.tensor.matmul(
                        ps[:, n * 512:(n + 1) * 512],
                        w8t[:, mb // 2, dk, :, (mb % 2) * P:(mb % 2 + 1) * P],
                        x8t[:, hh, dk, :, n * 512:(n + 1) * 512],
                        perf_mode=DR,
                        start=(dk == 0), stop=(dk == 3))
                    if dk == 3 and n == 1:
                        stag = stg.tile([P, 1024], FP8, name="stag")
                        nc.vector.tensor_copy(out=stag, in_=ps)
                        if mb == 2:
                            dup_q(stag, 2, hh, nc.gpsimd)
                        else:
                            dup_k(stag, 2, hh, nc.gpsimd)
                    return
                if it[0] == "p1":
                    _, _, mc, hh, kc, n = it
                    if kc == 0 and n == 0:
                        f_state["ps"] = ps_f_pool.tile([P, 1024], F32,
                                                       name="psf")
                    ps = f_state["ps"]
                    nc.tensor.matmul(
                        ps[:, n * 512:(n + 1) * 512],
                        wv_t[:, kc, mc * P:(mc + 1) * P],
                        xt[:, hh, kc, n * 512:(n + 1) * 512],
                        start=(kc == 0), stop=(kc == 7))
                    if kc == 7 and n == 1:
                        vt = vt01 if mc == 0 else vt23
                        nc.vector.tensor_copy(
                            out=vt[:, hh * 1024:(hh + 1) * 1024], in_=ps)
                else:
                    _, _, which, sc, n = it
                    if n == 0:
                        f_state["ps4"] = ps_f_pool.tile([P, 1024], F32,
                                                        name="psf")
                    ps4 = f_state["ps4"]
                    nc.tensor.matmul(
                        ps4[:, n * 512:(n + 1) * 512],
                        vt01[:, sc * P:(sc + 1) * P],
                        wo_t[:, 0, n * 512:(n + 1) * 512],
                        start=True, stop=True)
                    if n == 1:
                        o_sb = outp.tile([P, HIDDEN], BF16, name="osb")
                        nc.vector.tensor_copy(out=o_sb, in_=ps4)
                        nc.gpsimd.dma_start(
                            out=out2_d[sc * P:(sc + 1) * P, :], in_=o_sb)

            def pump(gc, budget):
                while budget > 0 and f_state["i"] < len(queue):
                    it = queue[f_state["i"]]
                    if it[1] > gc:
                        return
                    f_state["i"] += 1
                    emit_item(it)
                    budget -= 1

            def target(gc):
                return min(len(queue), int(2.5 * (gc + 1)) + 2)

            # -------- lead: q01 both S-halves, k01 both k-halves ----------
            def emit_lead(mb, hh, is_q, cast_eng, dma_eng):
                ps = ps_s_pool.tile([P, 1024], F32, name="pss")
                for dk in range(4):
                    for n in range(2):
                        nc.tensor.matmul(
                            ps[:, n * 512:(n + 1) * 512],
                            w8t[:, mb // 2, dk, :,
                                (mb % 2) * P:(mb % 2 + 1) * P],
                            x8t[:, hh, dk, :, n * 512:(n + 1) * 512],
                            perf_mode=DR,
                            start=(dk == 0), stop=(dk == 3))
                stag = stg.tile([P, 1024], FP8, name="stag")
                if cast_eng == "act":
                    nc.scalar.activation(stag, ps, COPY)
                else:
                    nc.vector.tensor_copy(out=stag, in_=ps)
                if is_q:
                    dup_q(stag, 0, hh, dma_eng)
                else:
                    dup_k(stag, 0, hh, dma_eng)

            emit_lead(0, 0, True, "act", nc.scalar)    # q01 S-half0
            emit_lead(1, 0, False, "vec", nc.gpsimd)   # k01 k-half0
            emit_lead(1, 1, False, "vec", nc.gpsimd)   # k01 k-half1
            emit_lead(0, 1, True, "vec", nc.gpsimd)    # q01 S-half1

            # ---------------- head loop ----------------
            def score_pair(j, qc, ps_tile, hh):
                qdj, kdj = qd[j], kd[j]
                rb = 64 * hh
                for n in range(2):
                    nc.tensor.matmul(
                        ps_tile[:, n * 512:(n + 1) * 512],
                        qdj[rb:rb + 64, qc * P:(qc + 1) * P],
                        kdj[rb:rb + 64, n * 512:(n + 1) * 512],
                        tile_position=(rb, 0),
                        start=True, stop=True)

            def matvec(j, half, pr, psc, first, last):
                eb, wb = e8[j % 2], wr8[j % 2]
                for n in range(2):
                    c0 = half * 1024 + n * 512
                    nc.tensor.matmul(
                        psc[:, n * 512:(n + 1) * 512],
                        wb[:, 2 * pr:2 * pr + 2, :],
                        eb[:, 2 * pr:2 * pr + 2, c0:c0 + 512],
                        perf_mode=DR,
                        start=first, stop=last)

            def vscale(j, half, psc):
                vt = vt01 if j < 2 else vt23
                bp = (j % 2) * 64
                c0 = half * 1024
                nc.vector.tensor_tensor(
                    vt[bp:bp + 64, c0:c0 + 1024], vt[bp:bp + 64, c0:c0 + 1024],
                    psc[bp:bp + 64, :], MULT)

            # first slot's hh0 scores, then steady state: emit hh0 of the
            # NEXT slot at the end of each slot.
            ps_next = ps_s_pool.tile([P, 1024], F32, name="pss")
            score_pair(0, 0, ps_next, 0)

            for j in range(HPC):
                eb, wb = e8[j % 2], wr8[j % 2]

                for qc in range(QC):
                    gc = j * QC + qc
                    ps_h0 = ps_next
                    ps_h1 = ps_s_pool.tile([P, 1024], F32, name="pss")
                    score_pair(j, qc, ps_h1, 1)
                    r_h = [None, None]
                    for hh in range(2):
                        r = rsp.tile([P, 1], F32, name=f"r{hh}")
                        nc.scalar.activation(
                            eb[:, qc, hh * 1024:(hh + 1) * 1024],
                            ps_h0 if hh == 0 else ps_h1,
                            EXP, scale=0.125, accum_out=r)
                        r_h[hh] = r
                        if hh == 0:
                            due = target(gc) - f_state["i"]
                            pump(gc, max(0, min(2, (due + 1) // 2)))
                    rs = rsp.tile([P, 1], F32, name="rs")
                    nc.vector.tensor_tensor(rs, r_h[0], r_h[1], ADD)
                    rinv = rsp.tile([P, 1], F32, name="rinv")
                    nc.vector.reciprocal(rinv, rs)
                    nc.vector.tensor_scalar(wb[:, qc, :],
                                            rinv.to_broadcast([P, P]),
                                            1024.0, None, MULT)
                    # next slot's hh0 scores (stream during exp(t, hh1)),
                    # emitted before matvec/filler so the exp pipeline
                    # never waits on the PE queue
                    if gc < HPC * QC - 1:
                        nj, nqc = (j, qc + 1) if qc + 1 < QC else (j + 1, 0)
                        ps_next = ps_s_pool.tile([P, 1024], F32, name="pss")
                        score_pair(nj, nqc, ps_next, 0)

                    # colsum matvec scheduling
                    if j < 3:
                        if j > 0 and qc < NPAIR:
                            if qc == 0:
                                f_state["psc"] = ps_c_pool.tile(
                                    [P, 1024], F32, name="psc")
                            matvec(j - 1, 1, qc, f_state["psc"],
                                   qc == 0, qc == NPAIR - 1)
                            if qc == NPAIR - 1:
                                vscale(j - 1, 1, f_state["psc"])
                        elif qc >= NPAIR:
                            pr = qc - NPAIR
                            if pr == 0:
                                f_state["psc"] = ps_c_pool.tile(
                                    [P, 1024], F32, name="psc")
                            matvec(j, 0, pr, f_state["psc"],
                                   pr == 0, pr == NPAIR - 1)
                            if pr == NPAIR - 1:
                                vscale(j, 0, f_state["psc"])
                    else:
                        if qc < NPAIR:
                            if qc == 0:
                                f_state["psc"] = ps_c_pool.tile(
                                    [P, 1024], F32, name="psc")
                            matvec(2, 1, qc, f_state["psc"],
                                   qc == 0, qc == NPAIR - 1)
                            if qc == NPAIR - 1:
                                vscale(2, 1, f_state["psc"])
                        elif qc >= 12:
                            # head 3: both colsum halves, 2 steps each/slot
                            if qc == 12:
                                f_state["psc"] = ps_c_pool.tile(
                                    [P, 1024], F32, name="psc")
                                f_state["psc2"] = ps_f_pool.tile(
                                    [P, 1024], F32, name="psf")
                            for t in range(2):
                                pr = (qc - 12) * 2 + t
                                matvec(3, 0, pr, f_state["psc"],
                                       pr == 0, pr == NPAIR - 1)
                                matvec(3, 1, pr, f_state["psc2"],
                                       pr == 0, pr == NPAIR - 1)
                    pump(gc, max(0, min(3, target(gc) - f_state["i"])))

            # ------- tail: vscale head3 + merged out3 (v23 full, K=128) ----
            vscale(3, 0, f_state["psc"])
            vscale(3, 1, f_state["psc2"])
            pump(63, len(queue))
            for sc in range(QC):
                ps_o = ps_s_pool.tile([P, 1024], F32, name="pss")
                for n in range(2):
                    nc.tensor.matmul(
                        ps_o[:, n * 512:(n + 1) * 512],
                        vt23[:, sc * P:(sc + 1) * P],
                        wo_t[:, 1, n * 512:(n + 1) * 512],
                        start=True, stop=True)
                o_sb = outp.tile([P, HIDDEN], BF16, name="osb")
                if sc % 2 == 0:
                    nc.scalar.activation(o_sb, ps_o, COPY)
                else:
                    nc.vector.tensor_copy(out=o_sb, in_=ps_o)
                nc.gpsimd.dma_start(out=out3_d[sc * P:(sc + 1) * P, :],
                                    in_=o_sb)

    nc.compile()
    _CACHE["nc"] = nc
    return nc


def kernel(x: np.ndarray, w_qkv: np.ndarray, w_o: np.ndarray) -> np.ndarray:
    import ml_dtypes
    from concourse.bass_utils import run_bass_kernel_spmd

    nc = _build()

    def pack4d(a):
        # [1024, C] -> [128, 4, 2, C]: row dk*256 + i*128 + p -> [p, dk, i]
        cc = a.shape[1]
        return np.ascontiguousarray(
            a.reshape(4, 2, 128, cc).transpose(2, 0, 1, 3))

    def to_fp8(a):
        return np.clip(a, -240.0, 240.0).astype(ml_dtypes.float8_e4m3)

    xTs = [np.ascontiguousarray(x[b].T) for b in range(B)]
    # x8: [128, 2(S-half), 4, 2, 1024]
    x8 = [to_fp8(np.ascontiguousarray(
        xTs[b].reshape(4, 2, 128, 2, 1024).transpose(2, 3, 0, 1, 4)))
        for b in range(B)]
    # xT: [128, 2(S-half), 8, 1024]
    xbf = [np.ascontiguousarray(
        xTs[b].reshape(8, 128, 2, 1024).transpose(1, 2, 0, 3)).astype(
            ml_dtypes.bfloat16) for b in range(B)]
    in_maps = []
    for c in range(N_CORES):
        b, g = divmod(c, HPC)
        base = 256 * g
        q01 = w_qkv[:, base:base + 128]
        q23 = w_qkv[:, base + 128:base + 256]
        k01 = w_qkv[:, 1024 + base:1024 + base + 128]
        k23 = w_qkv[:, 1024 + base + 128:1024 + base + 256]
        v01 = w_qkv[:, 2048 + base:2048 + base + 128]
        v23 = w_qkv[:, 2048 + base + 128:2048 + base + 256]
        wqk = np.concatenate([q01, k01, q23, k23], axis=1)
        # [1024, 512] -> [128, 2(group), 4, 2, 256]
        w8p = pack4d(wqk).reshape(128, 4, 2, 2, 256).transpose(
            0, 3, 1, 2, 4)
        wv = np.concatenate([v01, v23], axis=1)
        wo_slice = (w_o[base:base + 256, :] * (1.0 / 1024.0)).reshape(
            2, 128, HIDDEN).transpose(1, 0, 2)
        in_maps.append({
            "x8": x8[b],
            "w8": to_fp8(np.ascontiguousarray(w8p)),
            "xT": xbf[b],
            "wv": np.ascontiguousarray(
                wv.reshape(8, 128, 256).transpose(1, 0, 2)).astype(
                    ml_dtypes.bfloat16),
            "wo": np.ascontiguousarray(wo_slice).astype(ml_dtypes.bfloat16),
        })

    res = run_bass_kernel_spmd(nc, in_maps, list(range(N_CORES)),
                               **_CACHE.get("run_kwargs", {}))
    _CACHE["last_result"] = res

    out = np.zeros((B, S, HIDDEN), np.float32)
    for c in range(N_CORES):
        r = res.results[c]
        out[c // HPC] += (r["out2"].astype(np.float32)
                          + r["out3"].astype(np.float32))
    return out


# revision 10
# speedup vs baseline: 1.0448x; 1.0407x over previous
"""Trainium2 Bass kernel for nn_BaselineAttention_36172214567310 (v11).

Reference computation (einsum 'bhqk,bhkd->bhkd' sums over q, so attention
collapses to: v scaled by softmax column-sums):

    qkv = x @ w_qkv
    P = softmax(q @ k^T / 8)      per head, rows sum to 1
    colsum[k] = sum_q P[q, k]
    out = (v * colsum[:, None]) @ w_o

Sharding: 8 cores = 2 batches x 4 head-groups (4 heads each).

v11 = v10 with the lead and exp-pipeline stalls fixed (v10 trace: first
exp at 30.6us from serialized strided DMA issues; exps waited ~550ns on
score matmuls queued behind filler):
- x8/xT in S-half-major layout -> each input DMA is contiguous per
  partition (issue cost tracks descriptor count).
- kd stored RESHAPED [k-half0 rows 0-63 | k-half1 rows 64-127] (no
  duplication): 2 DMAs per projection half instead of 4.
- lead dup-DMA issues split across the scalar DGE (q01, before its
  exps) and gpsimd DGE (k01, q01-Sh1); q01-Sh1 moved into the lead.
- next slot's hh0 score pair is emitted at the END of the previous
  slot, so it streams during exp(t, hh1) instead of queueing behind
  matvec/filler matmuls.
- scores stay fp8 row-tile packed; q/k projection fp8-DR; v bf16;
  out3 = (v23 full, K=128) in the tail.
"""

import sys

sys.path.insert(0, "/opt/trn_rl_repo")

import numpy as np

B, S, HIDDEN = 2, 2048, 1024
NH, HD = 16, 64
HPC = 4
N_CORES = 8
P = 128
QC = S // P
NPAIR = QC // 2

_CACHE = {}


def _build():
    if "nc" in _CACHE:
        return _CACHE["nc"]

    import concourse.mybir as mybir
    import concourse.tile as tile
    from concourse import bacc

    F32 = mybir.dt.float32
    BF16 = mybir.dt.bfloat16
    FP8 = mybir.dt.float8e4
    EXP = mybir.ActivationFunctionType.Exp
    COPY = mybir.ActivationFunctionType.Copy
    ADD = mybir.AluOpType.add
    MULT = mybir.AluOpType.mult
    DR = mybir.MatmulPerfMode.DoubleRow

    nc = bacc.Bacc()
    x8_d = nc.declare_dram_parameter("x8", [P, 2, 4, 2, 1024], FP8,
                                     isOutput=False)
    w8_d = nc.declare_dram_parameter("w8", [P, 2, 4, 2, 256], FP8,
                                     isOutput=False)
    xT_d = nc.declare_dram_parameter("xT", [P, 2, 8, 1024], BF16,
                                     isOutput=False)
    wv_d = nc.declare_dram_parameter("wv", [P, 8, 256], BF16, isOutput=False)
    wo_d = nc.declare_dram_parameter("wo", [P, 2, HIDDEN], BF16,
                                     isOutput=False)
    out2_d = nc.declare_dram_parameter("out2", [S, HIDDEN], BF16,
                                       isOutput=True)
    out3_d = nc.declare_dram_parameter("out3", [S, HIDDEN], BF16,
                                       isOutput=True)

    with tile.TileContext(nc) as tc:
        with tc.tile_pool(name="persist", bufs=1) as sb, \
             tc.tile_pool(name="small", bufs=1) as sm, \
             tc.tile_pool(name="stag", bufs=2) as stg, \
             tc.tile_pool(name="rsp", bufs=8) as rsp, \
             tc.tile_pool(name="outp", bufs=3) as outp, \
             tc.tile_pool(name="ps_s", bufs=2, space="PSUM") as ps_s_pool, \
             tc.tile_pool(name="ps_c", bufs=1, space="PSUM") as ps_c_pool, \
             tc.tile_pool(name="ps_f", bufs=1, space="PSUM") as ps_f_pool:

            x8t = sb.tile([P, 2, 4, 2, 1024], FP8, name="x8t")
            w8t = sb.tile([P, 2, 4, 2, 256], FP8, name="w8t")
            xt = sb.tile([P, 2, 8, 1024], BF16, name="xt")
            wv_t = sb.tile([P, 8, 256], BF16, name="wvt")
            wo_t = sb.tile([P, 2, HIDDEN], BF16, name="wot")
            vt01 = sb.tile([P, S], BF16, name="vt01")
            vt23 = sb.tile([P, S], BF16, name="vt23")
            # q01/k01/q23/k23 bf16 [2 heads x 64 d-rows, S]
            qkt = [sb.tile([P, S], BF16, name=f"qkt{m}") for m in range(4)]
            e8 = [sb.tile([P, QC, S], FP8, name=f"e8_{i}") for i in range(2)]
            wr8 = [sb.tile([P, QC, P], FP8, name=f"wr8_{i}") for i in range(2)]
            dum = sm.tile([P, 1], F32, name="dum")
            dum2 = sm.tile([P, 1], F32, name="dum2")
            wsrc = sm.tile([P, 512], BF16, name="wsrc")

            # exp table preload + PE warmup (no DMA dependency)
            nc.vector.memset(dum, 0.0)
            nc.scalar.activation(dum2, dum, EXP)
            nc.vector.memset(wsrc, 0.0)
            wps = ps_c_pool.tile([P, 1024], F32, name="psc")
            for i in range(20):
                nc.tensor.matmul(wps[:, 0:512], wsrc[:, 0:128], wsrc,
                                 start=True, stop=True)

            # input DMA: lead-critical first; all contiguous per partition
            nc.sync.dma_start(out=w8t[:, 0], in_=w8_d[0:P, 0])
            nc.sync.dma_start(out=x8t[:, 0], in_=x8_d[0:P, 0])
            nc.sync.dma_start(out=w8t[:, 1], in_=w8_d[0:P, 1])
            nc.sync.dma_start(out=x8t[:, 1], in_=x8_d[0:P, 1])
            nc.sync.dma_start(out=xt[:, 0], in_=xT_d[0:P, 0])
            nc.sync.dma_start(out=wv_t, in_=wv_d[0:P, :, :])
            nc.sync.dma_start(out=xt[:, 1], in_=xT_d[0:P, 1])
            nc.sync.dma_start(out=wo_t, in_=wo_d[0:P, :, :])

            # ---------------- global filler queue ----------------
            # "qk8": one fp8-DR 512-col MM of the q/k projection (mb: 2=q23
            #   3=k23; hh = S-half for q, k-half for k; 4 dk accum steps);
            #   last step: cast to fp8 staging (DVE), dup/reshape-DMA.
            # "p1": one bf16 512-col MM of the v projection (mc 0=v01 1=v23).
            # "p4": out2 = v01 x wo0 (K=128) projection MM.
            queue = []
            for hh in range(2):                      # v01 (deadlines gc15/23)
                for kc in range(8):
                    for n in range(2):
                        queue.append(("p1", 0, 0, hh, kc, n))
            for mb in (2, 3):                        # q23, k23 (deadline gc30)
                for hh in range(2):
                    for dk in range(4):
                        for n in range(2):
                            queue.append(("qk8", 0, mb, hh, dk, n))
            for hh in range(2):                      # v23 (deadlines gc47/55)
                for kc in range(8):
                    for n in range(2):
                        queue.append(("p1", 0, 1, hh, kc, n))
            for sc in range(8):                      # A sc0-7: vscale(1,0)@31
                for n in range(2):
                    queue.append(("p4", 32, "A", sc, n))
            for sc in range(8, QC):                  # A sc8-15: vscale(1,1)@39
                for n in range(2):
                    queue.append(("p4", 40, "A", sc, n))

            f_state = {"i": 0, "ps": None, "ps4": None}

            def emit_item(it):
                if it[0] == "qk8":
                    _, _, mb, hh, dk, n = it
                    if dk == 0 and n == 0:
                        f_state["ps"] = ps_f_pool.tile([P, 1024], F32,
                                                       name="psf")
                    ps = f_state["ps"]
                    nc.tensor.matmul(
                        ps[:, n * 512:(n + 1) * 512],
                        w8t[:, mb // 2, dk, :, (mb % 2) * P:(mb % 2 + 1) * P],
                        x8t[:, hh, dk, :, n * 512:(n + 1) * 512],
                        perf_mode=DR,
                        start=(dk == 0), stop=(dk == 3))
                    if dk == 3 and n == 1:
                        nc.vector.tensor_copy(
                            out=qkt[mb][:, hh * 1024:(hh + 1) * 1024], in_=ps)
                    return
                if it[0] == "p1":
                    _, _, mc, hh, kc, n = it
                    if kc == 0 and n == 0:
                        f_state["ps"] = ps_f_pool.tile([P, 1024], F32,
                                                       name="psf")
                    ps = f_state["ps"]
                    nc.tensor.matmul(
                        ps[:, n * 512:(n + 1) * 512],
                        wv_t[:, kc, mc * P:(mc + 1) * P],
                        xt[:, hh, kc, n * 512:(n + 1) * 512],
                        start=(kc == 0), stop=(kc == 7))
                    if kc == 7 and n == 1:
                        vt = vt01 if mc == 0 else vt23
                        nc.vector.tensor_copy(
                            out=vt[:, hh * 1024:(hh + 1) * 1024], in_=ps)
                else:
                    _, _, which, sc, n = it
                    if n == 0:
                        f_state["ps4"] = ps_f_pool.tile([P, 1024], F32,
                                                        name="psf")
                    ps4 = f_state["ps4"]
                    nc.tensor.matmul(
                        ps4[:, n * 512:(n + 1) * 512],
                        vt01[:, sc * P:(sc + 1) * P],
                        wo_t[:, 0, n * 512:(n + 1) * 512],
                        start=True, stop=True)
                    if n == 1:
                        o_sb = outp.tile([P, HIDDEN], BF16, name="osb")
                        nc.vector.tensor_copy(out=o_sb, in_=ps4)
                        nc.gpsimd.dma_start(
                            out=out2_d[sc * P:(sc + 1) * P, :], in_=o_sb)

            def pump(gc, budget):
                while budget > 0 and f_state["i"] < len(queue):
                    it = queue[f_state["i"]]
                    if it[1] > gc:
                        return
                    f_state["i"] += 1
                    emit_item(it)
                    budget -= 1

            def target(gc):
                return min(len(queue), int(2.5 * (gc + 1)) + 2)

            # -------- lead: q01 both S-halves, k01 both k-halves ----------
            def emit_lead(mb, hh, is_q, cast_eng, dma_eng):
                ps = ps_s_pool.tile([P, 1024], F32, name="pss")
                for dk in range(4):
                    for n in range(2):
                        nc.tensor.matmul(
                            ps[:, n * 512:(n + 1) * 512],
                            w8t[:, mb // 2, dk, :,
                                (mb % 2) * P:(mb % 2 + 1) * P],
                            x8t[:, hh, dk, :, n * 512:(n + 1) * 512],
                            perf_mode=DR,
                            start=(dk == 0), stop=(dk == 3))
                dst = qkt[mb][:, hh * 1024:(hh + 1) * 1024]
                if cast_eng == "act":
                    nc.scalar.activation(dst, ps, COPY)
                else:
                    nc.vector.tensor_copy(out=dst, in_=ps)

            emit_lead(0, 0, True, "act", None)    # q01 S-half0
            emit_lead(1, 0, False, "vec", None)   # k01 S-half0
            emit_lead(1, 1, False, "vec", None)   # k01 S-half1
            emit_lead(0, 1, True, "vec", None)    # q01 S-half1

            # ---------------- head loop ----------------
            def score_pair(j, qc, ps_tile, hh):
                qt = qkt[0] if j < 2 else qkt[2]
                kt = qkt[1] if j < 2 else qkt[3]
                bp = (j % 2) * 64
                for n in range(2):
                    c0 = hh * 1024 + n * 512
                    nc.tensor.matmul(
                        ps_tile[:, n * 512:(n + 1) * 512],
                        qt[bp:bp + 64, qc * P:(qc + 1) * P],
                        kt[bp:bp + 64, c0:c0 + 512],
                        start=True, stop=True)

            def matvec(j, half, pr, psc, first, last):
                eb, wb = e8[j % 2], wr8[j % 2]
                for n in range(2):
                    c0 = half * 1024 + n * 512
                    nc.tensor.matmul(
                        psc[:, n * 512:(n + 1) * 512],
                        wb[:, 2 * pr:2 * pr + 2, :],
                        eb[:, 2 * pr:2 * pr + 2, c0:c0 + 512],
                        perf_mode=DR,
                        start=first, stop=last)

            def vscale(j, half, psc):
                vt = vt01 if j < 2 else vt23
                bp = (j % 2) * 64
                c0 = half * 1024
                nc.vector.tensor_tensor(
                    vt[bp:bp + 64, c0:c0 + 1024], vt[bp:bp + 64, c0:c0 + 1024],
                    psc[bp:bp + 64, :], MULT)

            # first slot's hh0 scores, then steady state: emit hh0 of the
            # NEXT slot at the end of each slot.
            ps_next = ps_s_pool.tile([P, 1024], F32, name="pss")
            score_pair(0, 0, ps_next, 0)

            for j in range(HPC):
                eb, wb = e8[j % 2], wr8[j % 2]

                for qc in range(QC):
                    gc = j * QC + qc
                    ps_h0 = ps_next
                    ps_h1 = ps_s_pool.tile([P, 1024], F32, name="pss")
                    score_pair(j, qc, ps_h1, 1)
                    r_h = [None, None]
                    for hh in range(2):
                        r = rsp.tile([P, 1], F32, name=f"r{hh}")
                        nc.scalar.activation(
                            eb[:, qc, hh * 1024:(hh + 1) * 1024],
                            ps_h0 if hh == 0 else ps_h1,
                            EXP, scale=0.125, accum_out=r)
                        r_h[hh] = r
                        if hh == 0:
                            due = target(gc) - f_state["i"]
                            pump(gc, max(0, min(2, (due + 1) // 2)))
                    rs = rsp.tile([P, 1], F32, name="rs")
                    nc.vector.tensor_tensor(rs, r_h[0], r_h[1], ADD)
                    rinv = rsp.tile([P, 1], F32, name="rinv")
                    nc.vector.reciprocal(rinv, rs)
                    nc.vector.tensor_scalar(wb[:, qc, :],
                                            rinv.to_broadcast([P, P]),
                                            1024.0, None, MULT)
                    # next slot's hh0 scores (stream during exp(t, hh1)),
                    # emitted before matvec/filler so the exp pipeline
                    # never waits on the PE queue
                    if gc < HPC * QC - 1:
                        nj, nqc = (j, qc + 1) if qc + 1 < QC else (j + 1, 0)
                        ps_next = ps_s_pool.tile([P, 1024], F32, name="pss")
                        score_pair(nj, nqc, ps_next, 0)

                    # colsum matvec scheduling
                    if j < 3:
                        if j > 0 and qc < NPAIR:
                            if qc == 0:
                                f_state["psc"] = ps_c_pool.tile(
                                    [P, 1024], F32, name="psc")
                            matvec(j - 1, 1, qc, f_state["psc"],
                                   qc == 0, qc == NPAIR - 1)
                            if qc == NPAIR - 1:
                                vscale(j - 1, 1, f_state["psc"])
                        elif qc >= NPAIR:
                            pr = qc - NPAIR
                            if pr == 0:
                                f_state["psc"] = ps_c_pool.tile(
                                    [P, 1024], F32, name="psc")
                            matvec(j, 0, pr, f_state["psc"],
                                   pr == 0, pr == NPAIR - 1)
                            if pr == NPAIR - 1:
                                vscale(j, 0, f_state["psc"])
                    else:
                        if qc < NPAIR:
                            if qc == 0:
                                f_state["psc"] = ps_c_pool.tile(
                                    [P, 1024], F32, name="psc")
                            matvec(2, 1, qc, f_state["psc"],
                                   qc == 0, qc == NPAIR - 1)
                            if qc == NPAIR - 1:
                                vscale(2, 1, f_state["psc"])
                        elif qc >= 12:
                            # head 3: both colsum halves, 2 steps each/slot
                            if qc == 12:
                                f_state["psc"] = ps_c_pool.tile(
                                    [P, 1024], F32, name="psc")
                                f_state["psc2"] = ps_f_pool.tile(
                                    [P, 1024], F32, name="psf")
                            for t in range(2):
                                pr = (qc - 12) * 2 + t
                                matvec(3, 0, pr, f_state["psc"],
                                       pr == 0, pr == NPAIR - 1)
                                matvec(3, 1, pr, f_state["psc2"],
                                       pr == 0, pr == NPAIR - 1)
                    pump(gc, max(0, min(3, target(gc) - f_state["i"])))

            # ------- tail: vscale head3 + merged out3 (v23 full, K=128) ----
            vscale(3, 0, f_state["psc"])
            vscale(3, 1, f_state["psc2"])
            pump(63, len(queue))
            for sc in range(QC):
                ps_o = ps_s_pool.tile([P, 1024], F32, name="pss")
                for n in range(2):
                    nc.tensor.matmul(
                        ps_o[:, n * 512:(n + 1) * 512],
                        vt23[:, sc * P:(sc + 1) * P],
                        wo_t[:, 1, n * 512:(n + 1) * 512],
                        start=True, stop=True)
                o_sb = outp.tile([P, HIDDEN], BF16, name="osb")
                if sc % 2 == 0:
                    nc.scalar.activation(o_sb, ps_o, COPY)
                else:
                    nc.vector.tensor_copy(out=o_sb, in_=ps_o)
                nc.gpsimd.dma_start(out=out3_d[sc * P:(sc + 1) * P, :],
                                    in_=o_sb)

    nc.compile()
    _CACHE["nc"] = nc
    return nc


def kernel(x: np.ndarray, w_qkv: np.ndarray, w_o: np.ndarray) -> np.ndarray:
    import ml_dtypes
    from concourse.bass_utils import run_bass_kernel_spmd

    nc = _build()

    def pack4d(a):
        # [1024, C] -> [128, 4, 2, C]: row dk*256 + i*128 + p -> [p, dk, i]
        cc = a.shape[1]
        return np.ascontiguousarray(
            a.reshape(4, 2, 128, cc).transpose(2, 0, 1, 3))

    def to_fp8(a):
        return np.clip(a, -240.0, 240.0).astype(ml_dtypes.float8_e4m3)

    xTs = [np.ascontiguousarray(x[b].T) for b in range(B)]
    # x8: [128, 2(S-half), 4, 2, 1024]
    x8 = [to_fp8(np.ascontiguousarray(
        xTs[b].reshape(4, 2, 128, 2, 1024).transpose(2, 3, 0, 1, 4)))
        for b in range(B)]
    # xT: [128, 2(S-half), 8, 1024]
    xbf = [np.ascontiguousarray(
        xTs[b].reshape(8, 128, 2, 1024).transpose(1, 2, 0, 3)).astype(
            ml_dtypes.bfloat16) for b in range(B)]
    in_maps = []
    for c in range(N_CORES):
        b, g = divmod(c, HPC)
        base = 256 * g
        q01 = w_qkv[:, base:base + 128]
        q23 = w_qkv[:, base + 128:base + 256]
        k01 = w_qkv[:, 1024 + base:1024 + base + 128]
        k23 = w_qkv[:, 1024 + base + 128:1024 + base + 256]
        v01 = w_qkv[:, 2048 + base:2048 + base + 128]
        v23 = w_qkv[:, 2048 + base + 128:2048 + base + 256]
        wqk = np.concatenate([q01, k01, q23, k23], axis=1)
        # [1024, 512] -> [128, 2(group), 4, 2, 256]
        w8p = pack4d(wqk).reshape(128, 4, 2, 2, 256).transpose(
            0, 3, 1, 2, 4)
        wv = np.concatenate([v01, v23], axis=1)
        wo_slice = (w_o[base:base + 256, :] * (1.0 / 1024.0)).reshape(
            2, 128, HIDDEN).transpose(1, 0, 2)
        in_maps.append({
            "x8": x8[b],
            "w8": to_fp8(np.ascontiguousarray(w8p)),
            "xT": xbf[b],
            "wv": np.ascontiguousarray(
                wv.reshape(8, 128, 256).transpose(1, 0, 2)).astype(
                    ml_dtypes.bfloat16),
            "wo": np.ascontiguousarray(wo_slice).astype(ml_dtypes.bfloat16),
        })

    res = run_bass_kernel_spmd(nc, in_maps, list(range(N_CORES)),
                               **_CACHE.get("run_kwargs", {}))
    _CACHE["last_result"] = res

    out = np.zeros((B, S, HIDDEN), np.float32)
    for c in range(N_CORES):
        r = res.results[c]
        out[c // HPC] += (r["out2"].astype(np.float32)
                          + r["out3"].astype(np.float32))
    return out


# revision 11
# speedup vs baseline: 1.0876x; 1.0409x over previous
"""Trainium2 Bass kernel for nn_BaselineAttention_36172214567310 (v11).

Reference computation (einsum 'bhqk,bhkd->bhkd' sums over q, so attention
collapses to: v scaled by softmax column-sums):

    qkv = x @ w_qkv
    P = softmax(q @ k^T / 8)      per head, rows sum to 1
    colsum[k] = sum_q P[q, k]
    out = (v * colsum[:, None]) @ w_o

Sharding: 8 cores = 2 batches x 4 head-groups (4 heads each).

v11 = v10 with the lead and exp-pipeline stalls fixed (v10 trace: first
exp at 30.6us from serialized strided DMA issues; exps waited ~550ns on
score matmuls queued behind filler):
- x8/xT in S-half-major layout -> each input DMA is contiguous per
  partition (issue cost tracks descriptor count).
- kd stored RESHAPED [k-half0 rows 0-63 | k-half1 rows 64-127] (no
  duplication): 2 DMAs per projection half instead of 4.
- lead dup-DMA issues split across the scalar DGE (q01, before its
  exps) and gpsimd DGE (k01, q01-Sh1); q01-Sh1 moved into the lead.
- next slot's hh0 score pair is emitted at the END of the previous
  slot, so it streams during exp(t, hh1) instead of queueing behind
  matvec/filler matmuls.
- scores stay fp8 row-tile packed; q/k projection fp8-DR; v bf16;
  out3 = (v23 full, K=128) in the tail.
"""

import sys

sys.path.insert(0, "/opt/trn_rl_repo")

import numpy as np

B, S, HIDDEN = 2, 2048, 1024
NH, HD = 16, 64
HPC = 4
N_CORES = 8
P = 128
QC = S // P
NPAIR = QC // 2

_CACHE = {}


def _build():
    if "nc" in _CACHE:
        return _CACHE["nc"]

    import concourse.mybir as mybir
    import concourse.tile as tile
    from concourse import bacc

    F32 = mybir.dt.float32
    BF16 = mybir.dt.bfloat16
    FP8 = mybir.dt.float8e4
    EXP = mybir.ActivationFunctionType.Exp
    COPY = mybir.ActivationFunctionType.Copy
    ADD = mybir.AluOpType.add
    MULT = mybir.AluOpType.mult
    DR = mybir.MatmulPerfMode.DoubleRow

    nc = bacc.Bacc()
    x8_d = nc.declare_dram_parameter("x8", [P, 2, 4, 2, 1024], FP8,
                                     isOutput=False)
    w8_d = nc.declare_dram_parameter("w8", [P, 2, 4, 2, 256], FP8,
                                     isOutput=False)
    xT_d = nc.declare_dram_parameter("xT", [P, 2, 8, 1024], BF16,
                                     isOutput=False)
    wv_d = nc.declare_dram_parameter("wv", [P, 8, 256], BF16, isOutput=False)
    wo_d = nc.declare_dram_parameter("wo", [P, 2, HIDDEN], BF16,
                                     isOutput=False)
    out_d = nc.declare_dram_parameter("out", [S, HIDDEN], BF16, isOutput=True)
    out2_d = nc.declare_dram_parameter("out2", [S, HIDDEN], BF16,
                                       isOutput=True)
    out3_d = nc.declare_dram_parameter("out3", [S, HIDDEN], BF16,
                                       isOutput=True)

    with tile.TileContext(nc) as tc:
        with tc.tile_pool(name="persist", bufs=1) as sb, \
             tc.tile_pool(name="small", bufs=1) as sm, \
             tc.tile_pool(name="stag", bufs=2) as stg, \
             tc.tile_pool(name="rsp", bufs=8) as rsp, \
             tc.tile_pool(name="outp", bufs=3) as outp, \
             tc.tile_pool(name="ps_s", bufs=2, space="PSUM") as ps_s_pool, \
             tc.tile_pool(name="ps_c", bufs=1, space="PSUM") as ps_c_pool, \
             tc.tile_pool(name="ps_f", bufs=1, space="PSUM") as ps_f_pool:

            x8t = sb.tile([P, 2, 4, 2, 1024], FP8, name="x8t")
            w8t = sb.tile([P, 2, 4, 2, 256], FP8, name="w8t")
            xt = sb.tile([P, 2, 8, 1024], BF16, name="xt")
            wv_t = sb.tile([P, 8, 256], BF16, name="wvt")
            wo_t = sb.tile([P, 2, HIDDEN], BF16, name="wot")
            vt01 = sb.tile([P, S], BF16, name="vt01")
            vt23 = sb.tile([P, S], BF16, name="vt23")
            # q01/k01/q23/k23 bf16 [2 heads x 64 d-rows, S]
            qkt = [sb.tile([P, S], BF16, name=f"qkt{m}") for m in range(4)]
            e8 = [sb.tile([P, QC, S], FP8, name=f"e8_{i}") for i in range(2)]
            wr8 = [sb.tile([P, QC, P], FP8, name=f"wr8_{i}") for i in range(2)]
            dum = sm.tile([P, 1], F32, name="dum")
            dum2 = sm.tile([P, 1], F32, name="dum2")
            wsrc = sm.tile([P, 512], BF16, name="wsrc")

            # exp table preload + PE warmup (no DMA dependency)
            nc.vector.memset(dum, 0.0)
            nc.scalar.activation(dum2, dum, EXP)
            nc.vector.memset(wsrc, 0.0)
            wps = ps_c_pool.tile([P, 1024], F32, name="psc")
            for i in range(20):
                nc.tensor.matmul(wps[:, 0:512], wsrc[:, 0:128], wsrc,
                                 start=True, stop=True)

            # input DMA: lead-critical first; all contiguous per partition
            nc.sync.dma_start(out=w8t[:, 0], in_=w8_d[0:P, 0])
            nc.sync.dma_start(out=x8t[:, 0], in_=x8_d[0:P, 0])
            nc.sync.dma_start(out=w8t[:, 1], in_=w8_d[0:P, 1])
            nc.sync.dma_start(out=x8t[:, 1], in_=x8_d[0:P, 1])
            nc.sync.dma_start(out=xt[:, 0], in_=xT_d[0:P, 0])
            nc.sync.dma_start(out=wv_t, in_=wv_d[0:P, :, :])
            nc.sync.dma_start(out=xt[:, 1], in_=xT_d[0:P, 1])
            nc.sync.dma_start(out=wo_t, in_=wo_d[0:P, :, :])

            # ---------------- global filler queue ----------------
            # "qk8": one fp8-DR 512-col MM of the q/k projection (mb: 2=q23
            #   3=k23; hh = S-half for q, k-half for k; 4 dk accum steps);
            #   last step: cast to fp8 staging (DVE), dup/reshape-DMA.
            # "p1": one bf16 512-col MM of the v projection (mc 0=v01 1=v23).
            # "p4": out2 = v01 x wo0 (K=128) projection MM.
            queue = []
            for hh in range(2):                      # v01 (deadlines gc15/23)
                for kc in range(8):
                    for n in range(2):
                        queue.append(("p1", 0, 0, hh, kc, n))
            for mb in (2, 3):                        # q23, k23 (deadline gc30)
                for hh in range(2):
                    for dk in range(4):
                        for n in range(2):
                            queue.append(("qk8", 0, mb, hh, dk, n))
            for hh in range(2):                      # v23 (deadlines gc47/55)
                for kc in range(8):
                    for n in range(2):
                        queue.append(("p1", 0, 1, hh, kc, n))
            for sc in range(8):                      # A sc0-7: vscale(1,0)@31
                for n in range(2):
                    queue.append(("p4", 32, "A", sc, n))
            for sc in range(8, QC):                  # A sc8-15: vscale(1,1)@39
                for n in range(2):
                    queue.append(("p4", 40, "A", sc, n))
            for sc in range(8):                      # B sc0-7: vscale(2,0)@47
                for n in range(2):
                    queue.append(("p4", 48, "B", sc, n))
            for sc in range(8, QC):                  # B sc8-15: vscale(2,1)@55
                for n in range(2):
                    queue.append(("p4", 56, "B", sc, n))

            f_state = {"i": 0, "ps": None, "ps4": None}

            def emit_item(it):
                if it[0] == "qk8":
                    _, _, mb, hh, dk, n = it
                    if dk == 0 and n == 0:
                        f_state["ps"] = ps_f_pool.tile([P, 1024], F32,
                                                       name="psf")
                    ps = f_state["ps"]
                    nc.tensor.matmul(
                        ps[:, n * 512:(n + 1) * 512],
                        w8t[:, mb // 2, dk, :, (mb % 2) * P:(mb % 2 + 1) * P],
                        x8t[:, hh, dk, :, n * 512:(n + 1) * 512],
                        perf_mode=DR,
                        start=(dk == 0), stop=(dk == 3))
                    if dk == 3 and n == 1:
                        nc.vector.tensor_copy(
                            out=qkt[mb][:, hh * 1024:(hh + 1) * 1024], in_=ps)
                    return
                if it[0] == "p1":
                    _, _, mc, hh, kc, n = it
                    if kc == 0 and n == 0:
                        f_state["ps"] = ps_f_pool.tile([P, 1024], F32,
                                                       name="psf")
                    ps = f_state["ps"]
                    nc.tensor.matmul(
                        ps[:, n * 512:(n + 1) * 512],
                        wv_t[:, kc, mc * P:(mc + 1) * P],
                        xt[:, hh, kc, n * 512:(n + 1) * 512],
                        start=(kc == 0), stop=(kc == 7))
                    if kc == 7 and n == 1:
                        vt = vt01 if mc == 0 else vt23
                        nc.vector.tensor_copy(
                            out=vt[:, hh * 1024:(hh + 1) * 1024], in_=ps)
                else:
                    _, _, which, sc, n = it
                    if n == 0:
                        f_state["ps4"] = ps_f_pool.tile([P, 1024], F32,
                                                        name="psf")
                    ps4 = f_state["ps4"]
                    if which == "A":
                        lhsT = vt01[:, sc * P:(sc + 1) * P]
                        rhs = wo_t[:, 0, n * 512:(n + 1) * 512]
                        dst = out2_d
                    else:
                        lhsT = vt23[0:64, sc * P:(sc + 1) * P]
                        rhs = wo_t[0:64, 1, n * 512:(n + 1) * 512]
                        dst = out3_d
                    nc.tensor.matmul(ps4[:, n * 512:(n + 1) * 512], lhsT, rhs,
                                     start=True, stop=True)
                    if n == 1:
                        o_sb = outp.tile([P, HIDDEN], BF16, name="osb")
                        nc.vector.tensor_copy(out=o_sb, in_=ps4)
                        nc.gpsimd.dma_start(
                            out=dst[sc * P:(sc + 1) * P, :], in_=o_sb)

            def pump(gc, budget):
                while budget > 0 and f_state["i"] < len(queue):
                    it = queue[f_state["i"]]
                    if it[1] > gc:
                        return
                    f_state["i"] += 1
                    emit_item(it)
                    budget -= 1

            def target(gc):
                if gc < 8:
                    return 4 * (gc + 1)
                if gc < 40:
                    return 32 + 3 * (gc - 7)
                return min(len(queue), 128 + 3 * (gc - 39))

            # -------- lead: q01 both S-halves, k01 both k-halves ----------
            def emit_lead(mb, hh, is_q, cast_eng, dma_eng):
                ps = ps_s_pool.tile([P, 1024], F32, name="pss")
                for dk in range(4):
                    for n in range(2):
                        nc.tensor.matmul(
                            ps[:, n * 512:(n + 1) * 512],
                            w8t[:, mb // 2, dk, :,
                                (mb % 2) * P:(mb % 2 + 1) * P],
                            x8t[:, hh, dk, :, n * 512:(n + 1) * 512],
                            perf_mode=DR,
                            start=(dk == 0), stop=(dk == 3))
                dst = qkt[mb][:, hh * 1024:(hh + 1) * 1024]
                if cast_eng == "act":
                    nc.scalar.activation(dst, ps, COPY)
                else:
                    nc.vector.tensor_copy(out=dst, in_=ps)

            emit_lead(0, 0, True, "act", None)    # q01 S-half0
            emit_lead(1, 0, False, "vec", None)   # k01 S-half0
            emit_lead(1, 1, False, "vec", None)   # k01 S-half1
            emit_lead(0, 1, True, "vec", None)    # q01 S-half1

            # ---------------- head loop ----------------
            def score_pair(j, qc, ps_tile, hh):
                qt = qkt[0] if j < 2 else qkt[2]
                kt = qkt[1] if j < 2 else qkt[3]
                bp = (j % 2) * 64
                for n in range(2):
                    c0 = hh * 1024 + n * 512
                    nc.tensor.matmul(
                        ps_tile[:, n * 512:(n + 1) * 512],
                        qt[bp:bp + 64, qc * P:(qc + 1) * P],
                        kt[bp:bp + 64, c0:c0 + 512],
                        start=True, stop=True)

            def matvec(j, half, pr, psc, first, last):
                eb, wb = e8[j % 2], wr8[j % 2]
                for n in range(2):
                    c0 = half * 1024 + n * 512
                    nc.tensor.matmul(
                        psc[:, n * 512:(n + 1) * 512],
                        wb[:, 2 * pr:2 * pr + 2, :],
                        eb[:, 2 * pr:2 * pr + 2, c0:c0 + 512],
                        perf_mode=DR,
                        start=first, stop=last)

            def vscale(j, half, psc):
                vt = vt01 if j < 2 else vt23
                bp = (j % 2) * 64
                c0 = half * 1024
                nc.vector.tensor_tensor(
                    vt[bp:bp + 64, c0:c0 + 1024], vt[bp:bp + 64, c0:c0 + 1024],
                    psc[bp:bp + 64, :], MULT)

            # first slot's hh0 scores, then steady state: emit hh0 of the
            # NEXT slot at the end of each slot.
            ps_next = ps_s_pool.tile([P, 1024], F32, name="pss")
            score_pair(0, 0, ps_next, 0)

            for j in range(HPC):
                eb, wb = e8[j % 2], wr8[j % 2]

                for qc in range(QC):
                    gc = j * QC + qc
                    ps_h0 = ps_next
                    ps_h1 = ps_s_pool.tile([P, 1024], F32, name="pss")
                    score_pair(j, qc, ps_h1, 1)
                    r_h = [None, None]
                    for hh in range(2):
                        r = rsp.tile([P, 1], F32, name=f"r{hh}")
                        nc.scalar.activation(
                            eb[:, qc, hh * 1024:(hh + 1) * 1024],
                            ps_h0 if hh == 0 else ps_h1,
                            EXP, scale=0.125, accum_out=r)
                        r_h[hh] = r
                        if hh == 0:
                            due = target(gc) - f_state["i"]
                            pump(gc, max(0, min(3, (due + 1) // 2)))
                    rs = rsp.tile([P, 1], F32, name="rs")
                    nc.vector.tensor_tensor(rs, r_h[0], r_h[1], ADD)
                    rinv = rsp.tile([P, 1], F32, name="rinv")
                    nc.vector.reciprocal(rinv, rs)
                    nc.vector.tensor_scalar(wb[:, qc, :],
                                            rinv.to_broadcast([P, P]),
                                            1024.0, None, MULT)
                    # next slot's hh0 scores (stream during exp(t, hh1)),
                    # emitted before matvec/filler so the exp pipeline
                    # never waits on the PE queue
                    if gc < HPC * QC - 1:
                        nj, nqc = (j, qc + 1) if qc + 1 < QC else (j + 1, 0)
                        ps_next = ps_s_pool.tile([P, 1024], F32, name="pss")
                        score_pair(nj, nqc, ps_next, 0)

                    # colsum matvec scheduling
                    if j < 3:
                        if j > 0 and qc < NPAIR:
                            if qc == 0:
                                f_state["psc"] = ps_c_pool.tile(
                                    [P, 1024], F32, name="psc")
                            matvec(j - 1, 1, qc, f_state["psc"],
                                   qc == 0, qc == NPAIR - 1)
                            if qc == NPAIR - 1:
                                vscale(j - 1, 1, f_state["psc"])
                        elif qc >= NPAIR:
                            pr = qc - NPAIR
                            if pr == 0:
                                f_state["psc"] = ps_c_pool.tile(
                                    [P, 1024], F32, name="psc")
                            matvec(j, 0, pr, f_state["psc"],
                                   pr == 0, pr == NPAIR - 1)
                            if pr == NPAIR - 1:
                                vscale(j, 0, f_state["psc"])
                    else:
                        if qc < NPAIR:
                            if qc == 0:
                                f_state["psc"] = ps_c_pool.tile(
                                    [P, 1024], F32, name="psc")
                            matvec(2, 1, qc, f_state["psc"],
                                   qc == 0, qc == NPAIR - 1)
                            if qc == NPAIR - 1:
                                vscale(2, 1, f_state["psc"])
                        elif qc >= 12:
                            # head 3: both colsum halves, 2 steps each/slot
                            if qc == 12:
                                f_state["psc"] = ps_c_pool.tile(
                                    [P, 1024], F32, name="psc")
                                f_state["psc2"] = ps_f_pool.tile(
                                    [P, 1024], F32, name="psf")
                            for t in range(2):
                                pr = (qc - 12) * 2 + t
                                matvec(3, 0, pr, f_state["psc"],
                                       pr == 0, pr == NPAIR - 1)
                                matvec(3, 1, pr, f_state["psc2"],
                                       pr == 0, pr == NPAIR - 1)
                    pump(gc, max(0, min(5, target(gc) - f_state["i"])))

            # ------- tail: vscale head3 + merged out3 (v23 full, K=128) ----
            vscale(3, 0, f_state["psc"])
            vscale(3, 1, f_state["psc2"])
            pump(63, len(queue))
            for sc in range(QC):
                ps_o = ps_s_pool.tile([P, 1024], F32, name="pss")
                for n in range(2):
                    nc.tensor.matmul(
                        ps_o[:, n * 512:(n + 1) * 512],
                        vt23[64:128, sc * P:(sc + 1) * P],
                        wo_t[64:128, 1, n * 512:(n + 1) * 512],
                        start=True, stop=True)
                o_sb = outp.tile([P, HIDDEN], BF16, name="osb")
                if sc % 2 == 0:
                    nc.scalar.activation(o_sb, ps_o, COPY)
                else:
                    nc.vector.tensor_copy(out=o_sb, in_=ps_o)
                nc.gpsimd.dma_start(out=out_d[sc * P:(sc + 1) * P, :],
                                    in_=o_sb)

    nc.compile()
    _CACHE["nc"] = nc
    return nc


def kernel(x: np.ndarray, w_qkv: np.ndarray, w_o: np.ndarray) -> np.ndarray:
    import ml_dtypes
    from concourse.bass_utils import run_bass_kernel_spmd

    nc = _build()

    def pack4d(a):
        # [1024, C] -> [128, 4, 2, C]: row dk*256 + i*128 + p -> [p, dk, i]
        cc = a.shape[1]
        return np.ascontiguousarray(
            a.reshape(4, 2, 128, cc).transpose(2, 0, 1, 3))

    def to_fp8(a):
        return np.clip(a, -240.0, 240.0).astype(ml_dtypes.float8_e4m3)

    xTs = [np.ascontiguousarray(x[b].T) for b in range(B)]
    # x8: [128, 2(S-half), 4, 2, 1024]
    x8 = [to_fp8(np.ascontiguousarray(
        xTs[b].reshape(4, 2, 128, 2, 1024).transpose(2, 3, 0, 1, 4)))
        for b in range(B)]
    # xT: [128, 2(S-half), 8, 1024]
    xbf = [np.ascontiguousarray(
        xTs[b].reshape(8, 128, 2, 1024).transpose(1, 2, 0, 3)).astype(
            ml_dtypes.bfloat16) for b in range(B)]
    in_maps = []
    for c in range(N_CORES):
        b, g = divmod(c, HPC)
        base = 256 * g
        q01 = w_qkv[:, base:base + 128]
        q23 = w_qkv[:, base + 128:base + 256]
        k01 = w_qkv[:, 1024 + base:1024 + base + 128]
        k23 = w_qkv[:, 1024 + base + 128:1024 + base + 256]
        v01 = w_qkv[:, 2048 + base:2048 + base + 128]
        v23 = w_qkv[:, 2048 + base + 128:2048 + base + 256]
        wqk = np.concatenate([q01, k01, q23, k23], axis=1)
        # [1024, 512] -> [128, 2(group), 4, 2, 256]
        w8p = pack4d(wqk).reshape(128, 4, 2, 2, 256).transpose(
            0, 3, 1, 2, 4)
        wv = np.concatenate([v01, v23], axis=1)
        wo_slice = (w_o[base:base + 256, :] * (1.0 / 1024.0)).reshape(
            2, 128, HIDDEN).transpose(1, 0, 2)
        in_maps.append({
            "x8": x8[b],
            "w8": to_fp8(np.ascontiguousarray(w8p)),
            "xT": xbf[b],
            "wv": np.ascontiguousarray(
                wv.reshape(8, 128, 256).transpose(1, 0, 2)).astype(
                    ml_dtypes.bfloat16),
            "wo": np.ascontiguousarray(wo_slice).astype(ml_dtypes.bfloat16),
        })

    res = run_bass_kernel_spmd(nc, in_maps, list(range(N_CORES)),
                               **_CACHE.get("run_kwargs", {}))
    _CACHE["last_result"] = res

    out = np.zeros((B, S, HIDDEN), np.float32)
    for c in range(N_CORES):
        r = res.results[c]
        out[c // HPC] += (r["out"].astype(np.float32)
                          + r["out2"].astype(np.float32)
                          + r["out3"].astype(np.float32))
    return out


# revision 12
# speedup vs baseline: 1.1072x; 1.0181x over previous
"""Trainium2 Bass kernel for nn_BaselineAttention_36172214567310 (v4).

Reference computation (einsum 'bhqk,bhkd->bhkd' sums over q, so attention
collapses to: v scaled by softmax column-sums):

    qkv = x @ w_qkv
    P = softmax(q @ k^T / 8)      per head, rows sum to 1
    colsum[k] = sum_q P[q, k]
    out = (v * colsum[:, None]) @ w_o

Sharding: 8 cores = 2 batches x 4 head-groups (4 heads each).

v7 = v6 - LDW padding (measured: hurts) + bf16 outputs (the three
partial outputs totalled 24MB fp32 of DMA writes and made the tail
DMA-bound; bf16 halves it, host sums in fp32).
v5 = v4 + q/k projections in fp8 DoubleRow (half the MMs, shorter lead).
v4: uniform PE instruction density to keep the HAM clock-gate warm:
- warmup MMs at t=0 (no DMA dependency) so the lead projection runs at
  2.4 GHz; lead is only q01-h0 + k01.
- one global filler queue (rest of the qkv projection, then the output
  projection in four readiness-gated quarter phases) paced at ~3.6
  instructions per chunk under the scores+exp+matvec steady loop.
- output projection is split into three DRAM outputs (v01 x wo0 k-half,
  v23-head2 rows, v23-head3 rows) summed on the host, so nearly all of
  P4 streams out mid-kernel.
- lagged DoubleRow fp8 matvec (no bursts): head j k-half1 runs during
  head j+1 chunks 0-7, k-half0 during own chunks 8-15.
"""

import sys

sys.path.insert(0, "/opt/trn_rl_repo")

import numpy as np

B, S, HIDDEN = 2, 2048, 1024
NH, HD = 16, 64
HPC = 4
N_CORES = 8
P = 128
QC = S // P
NPAIR = QC // 2

_CACHE = {}


def _build():
    if "nc" in _CACHE:
        return _CACHE["nc"]

    import concourse.mybir as mybir
    import concourse.tile as tile
    from concourse import bacc

    F32 = mybir.dt.float32
    BF16 = mybir.dt.bfloat16
    FP8 = mybir.dt.float8e4
    EXP = mybir.ActivationFunctionType.Exp
    COPY = mybir.ActivationFunctionType.Copy
    ADD = mybir.AluOpType.add
    MULT = mybir.AluOpType.mult
    DR = mybir.MatmulPerfMode.DoubleRow

    nc = bacc.Bacc()
    xT_d = nc.declare_dram_parameter("xT", [HIDDEN, S], BF16, isOutput=False)
    x8_d = nc.declare_dram_parameter("x8", [HIDDEN // 2, 2 * S], FP8, isOutput=False)
    w8_d = nc.declare_dram_parameter("w8", [HIDDEN // 2, 2 * 512], FP8, isOutput=False)
    wqkv_d = nc.declare_dram_parameter("wqkv", [HIDDEN, 256], BF16, isOutput=False)
    wo_d = nc.declare_dram_parameter("wo", [256, HIDDEN], BF16, isOutput=False)
    out_d = nc.declare_dram_parameter("out", [S, HIDDEN], BF16, isOutput=True)
    out2_d = nc.declare_dram_parameter("out2", [S, HIDDEN], BF16, isOutput=True)
    out3_d = nc.declare_dram_parameter("out3", [S, HIDDEN], BF16, isOutput=True)

    with tile.TileContext(nc) as tc:
        with tc.tile_pool(name="persist", bufs=1) as sb, \
             tc.tile_pool(name="small", bufs=1) as sm, \
             tc.tile_pool(name="rsp", bufs=8) as rsp, \
             tc.tile_pool(name="outp", bufs=3) as outp, \
             tc.tile_pool(name="ps_s", bufs=2, space="PSUM") as ps_s_pool, \
             tc.tile_pool(name="ps_c", bufs=1, space="PSUM") as ps_c_pool, \
             tc.tile_pool(name="ps_f", bufs=1, space="PSUM") as ps_f_pool:

            xt = [sb.tile([P, S], BF16, name=f"xt{kc}") for kc in range(8)]
            x8t = [sb.tile([P, 2, S], FP8, name=f"x8t{p}") for p in range(4)]
            w8t = [sb.tile([P, 2, 512], FP8, name=f"w8t{p}") for p in range(4)]
            wq_t = [sb.tile([P, 256], BF16, name=f"wq{kc}") for kc in range(8)]
            wo_t = [sb.tile([P, HIDDEN], BF16, name=f"wo{kc}") for kc in range(2)]
            qkvt = [sb.tile([P, S], BF16, name=f"qkvt{mc}") for mc in range(6)]
            e8 = [sb.tile([P, QC, S], FP8, name=f"e8_{i}") for i in range(2)]
            wr8 = [sb.tile([P, QC, P], FP8, name=f"wr8_{i}") for i in range(2)]
            dum = sm.tile([P, 1], F32, name="dum")
            dum2 = sm.tile([P, 1], F32, name="dum2")
            wsrc = sm.tile([P, 512], BF16, name="wsrc")

            # exp table preload + PE warmup (no DMA dependency)
            nc.vector.memset(dum, 0.0)
            nc.scalar.activation(dum2, dum, EXP)
            nc.vector.memset(wsrc, 0.0)
            wps = ps_c_pool.tile([P, 1024], F32, name="psc")
            for i in range(20):
                nc.tensor.matmul(wps[:, 0:512], wsrc[:, 0:128], wsrc,
                                 start=True, stop=True)

            for p in range(4):
                nc.sync.dma_start(out=w8t[p], in_=w8_d[p * P:(p + 1) * P, :])
                nc.sync.dma_start(out=x8t[p], in_=x8_d[p * P:(p + 1) * P, :])
            for kc in range(8):
                nc.sync.dma_start(out=wq_t[kc],
                                  in_=wqkv_d[kc * P:(kc + 1) * P, :])
                nc.sync.dma_start(out=xt[kc], in_=xT_d[kc * P:(kc + 1) * P, :])
            for kc in range(2):
                nc.sync.dma_start(out=wo_t[kc],
                                  in_=wo_d[kc * P:(kc + 1) * P, :])

            # ---------------- global filler queue ----------------
            # p1 item: ("p1", ready, mc, hh, kc, n) - one 512-col MM of the
            #   qkv projection (16 MMs per (mc,hh) accumulation + copy).
            # p4 item: ("p4", ready, which, sc, n) - output projection MM.
            queue = []
            for dk in range(4):                      # q01-h1 (DR), deadline gc 8
                for n in range(2):
                    queue.append(("qk", 0, 0, 1, dk, n))
            for mc in (2,):                          # v01 (bf16)
                for hh in range(2):
                    for kc in range(8):
                        for n in range(2):
                            queue.append(("p1", 0, mc, hh, kc, n))
            for mc in (1, 2):                        # q23, k23 (DR; m-block 2,3... mapped below)
                for hh in range(2):
                    for dk in range(4):
                        for n in range(2):
                            queue.append(("qk", 0, mc + 1, hh, dk, n))
            for mc in (5,):                          # v23 (bf16)
                for hh in range(2):
                    for kc in range(8):
                        for n in range(2):
                            queue.append(("p1", 0, mc, hh, kc, n))
            # p4 quarter phases:
            # A: out2 = v01^T x wo0            (full K=128), sc 0-7 ready 33,
            #    sc 8-15 ready 41
            # B: out3 = v23[head2 rows] x wo1  (K=64),      sc 0-7 ready 49,
            #    sc 8-15 ready 57
            for sc in range(8):
                for n in range(2):
                    queue.append(("p4", 33, "A", sc, n))
            for sc in range(8, QC):
                for n in range(2):
                    queue.append(("p4", 41, "A", sc, n))
            for sc in range(8):
                for n in range(2):
                    queue.append(("p4", 49, "B", sc, n))
            for sc in range(8, QC):
                for n in range(2):
                    queue.append(("p4", 57, "B", sc, n))

            f_state = {"i": 0, "ps": None, "ps4": None}

            def emit_item(it):
                if it[0] == "qk":
                    _, _, mb, hh, dk, n = it
                    if dk == 0 and n == 0:
                        f_state["ps"] = ps_f_pool.tile([P, 1024], F32,
                                                       name="psf")
                    ps = f_state["ps"]
                    c0 = hh * 1024 + n * 512
                    nc.tensor.matmul(
                        ps[:, n * 512:(n + 1) * 512],
                        w8t[dk][:, :, mb * P:(mb + 1) * P],
                        x8t[dk][:, :, c0:c0 + 512],
                        perf_mode=DR,
                        start=(dk == 0), stop=(dk == 3))
                    if dk == 3 and n == 1:
                        qdst = {0: 0, 2: 3, 3: 4}[mb]
                        nc.vector.tensor_copy(
                            out=qkvt[qdst][:, hh * 1024:(hh + 1) * 1024],
                            in_=ps)
                    return
                if it[0] == "p1":
                    _, _, mc, hh, kc, n = it
                    if kc == 0 and n == 0:
                        f_state["ps"] = ps_f_pool.tile([P, 1024], F32,
                                                       name="psf")
                    ps = f_state["ps"]
                    c0 = hh * 1024 + n * 512
                    wcol = 0 if mc == 2 else P
                    nc.tensor.matmul(
                        ps[:, n * 512:(n + 1) * 512],
                        wq_t[kc][:, wcol:wcol + P],
                        xt[kc][:, c0:c0 + 512],
                        start=(kc == 0), stop=(kc == 7))
                    if kc == 7 and n == 1:
                        nc.vector.tensor_copy(
                            out=qkvt[mc][:, hh * 1024:(hh + 1) * 1024], in_=ps)
                else:
                    _, _, which, sc, n = it
                    if n == 0:
                        f_state["ps4"] = ps_f_pool.tile([P, 1024], F32,
                                                        name="psf")
                    ps4 = f_state["ps4"]
                    if which == "A":
                        lhsT = qkvt[2][:, sc * P:(sc + 1) * P]
                        rhs = wo_t[0][:, n * 512:(n + 1) * 512]
                        dst = out2_d
                    else:
                        lhsT = qkvt[5][0:64, sc * P:(sc + 1) * P]
                        rhs = wo_t[1][0:64, n * 512:(n + 1) * 512]
                        dst = out3_d
                    nc.tensor.matmul(ps4[:, n * 512:(n + 1) * 512], lhsT, rhs,
                                     start=True, stop=True)
                    if n == 1:
                        o_sb = outp.tile([P, HIDDEN], BF16, name="osb")
                        nc.vector.tensor_copy(out=o_sb, in_=ps4)
                        nc.sync.dma_start(out=dst[sc * P:(sc + 1) * P, :],
                                          in_=o_sb)

            def pump(gc, budget):
                while budget > 0 and f_state["i"] < len(queue):
                    it = queue[f_state["i"]]
                    if it[1] > gc:
                        return
                    f_state["i"] += 1
                    emit_item(it)
                    budget -= 1

            def target(gc):
                if gc < 8:
                    return int(4.5 * (gc + 1))
                if gc <= 46:
                    return 36 + int(2.75 * (gc - 7))
                return min(len(queue), 137 + 2 * (gc - 46))

            # ---------------- lead: q01-h0, k01 ----------------
            def emit_lead(mb, qdst, hh, on_act):
                ps = ps_s_pool.tile([P, 1024], F32, name="pss")
                for dk in range(4):
                    for n in range(2):
                        c0 = hh * 1024 + n * 512
                        nc.tensor.matmul(
                            ps[:, n * 512:(n + 1) * 512],
                            w8t[dk][:, :, mb * P:(mb + 1) * P],
                            x8t[dk][:, :, c0:c0 + 512],
                            perf_mode=DR,
                            start=(dk == 0), stop=(dk == 3))
                dst = qkvt[qdst][:, hh * 1024:(hh + 1) * 1024]
                if on_act:
                    nc.scalar.activation(dst, ps, COPY)
                else:
                    nc.vector.tensor_copy(out=dst, in_=ps)

            emit_lead(0, 0, 0, True)
            emit_lead(1, 1, 0, False)
            emit_lead(1, 1, 1, False)

            # ---------------- head loop ----------------
            def matvec(j, half, pr, first, last):
                eb, wb = e8[j % 2], wr8[j % 2]
                psc = f_state["psc"]
                for n in range(2):
                    c0 = half * 1024 + n * 512
                    nc.tensor.matmul(
                        psc[:, n * 512:(n + 1) * 512],
                        wb[:, 2 * pr:2 * pr + 2, :],
                        eb[:, 2 * pr:2 * pr + 2, c0:c0 + 512],
                        perf_mode=DR,
                        start=first, stop=last)

            def vscale(j, half):
                vt = qkvt[2 if j < 2 else 5]
                bp = (j % 2) * 64
                psc = f_state["psc"]
                c0 = half * 1024
                nc.vector.tensor_tensor(
                    vt[bp:bp + 64, c0:c0 + 1024], vt[bp:bp + 64, c0:c0 + 1024],
                    psc[bp:bp + 64, :], MULT)

            for j in range(HPC):
                qt = qkvt[0 if j < 2 else 3]
                kt = qkvt[1 if j < 2 else 4]
                bp = (j % 2) * 64
                eb, wb = e8[j % 2], wr8[j % 2]

                for qc in range(QC):
                    gc = j * QC + qc
                    r_h = [None, None]
                    for hh in range(2):
                        ps_s = ps_s_pool.tile([P, 1024], F32, name="pss")
                        for n in range(2):
                            c0 = hh * 1024 + n * 512
                            nc.tensor.matmul(
                                ps_s[:, n * 512:(n + 1) * 512],
                                qt[bp:bp + 64, qc * P:(qc + 1) * P],
                                kt[bp:bp + 64, c0:c0 + 512],
                                start=True, stop=True)
                        r = rsp.tile([P, 1], F32, name=f"r{hh}")
                        nc.scalar.activation(
                            eb[:, qc, hh * 1024:(hh + 1) * 1024],
                            ps_s, EXP, scale=0.125, accum_out=r)
                        r_h[hh] = r
                        if hh == 0:
                            due = target(gc) - f_state["i"]
                            pump(gc, max(0, min(3, (due + 1) // 2)))
                    rs = rsp.tile([P, 1], F32, name="rs")
                    nc.vector.tensor_tensor(rs, r_h[0], r_h[1], ADD)
                    rinv = rsp.tile([P, 1], F32, name="rinv")
                    nc.vector.reciprocal(rinv, rs)
                    nc.vector.tensor_scalar(wb[:, qc, :],
                                            rinv.to_broadcast([P, P]),
                                            1024.0, None, MULT)
                    has_mv = (qc < NPAIR and j > 0) or qc >= NPAIR
                    if qc < NPAIR and j > 0:
                        if qc == 0:
                            f_state["psc"] = ps_c_pool.tile(
                                [P, 1024], F32, name="psc")
                        matvec(j - 1, 1, qc, qc == 0, qc == NPAIR - 1)
                        if qc == NPAIR - 1:
                            vscale(j - 1, 1)
                    elif qc >= NPAIR:
                        pr = qc - NPAIR
                        if pr == 0:
                            f_state["psc"] = ps_c_pool.tile(
                                [P, 1024], F32, name="psc")
                        matvec(j, 0, pr, pr == 0, pr == NPAIR - 1)
                        if pr == NPAIR - 1:
                            vscale(j, 0)
                    pump(gc, max(0, min(5, target(gc) - f_state["i"])))

            # ---------------- tail ----------------
            # head 3 k-half1 matvec interleaved with out += v23[head3] x wo1
            # for s-chunks 0-7 (those only need the k-half0 v-scale, done).
            def p4bb(sc):
                ps_o = ps_s_pool.tile([P, 1024], F32, name="pss")
                for n in range(2):
                    nc.tensor.matmul(
                        ps_o[:, n * 512:(n + 1) * 512],
                        qkvt[5][64:128, sc * P:(sc + 1) * P],
                        wo_t[1][64:128, n * 512:(n + 1) * 512],
                        start=True, stop=True)
                o_sb = outp.tile([P, HIDDEN], BF16, name="osb")
                if sc % 2 == 0:
                    nc.scalar.activation(o_sb, ps_o, COPY)
                else:
                    nc.vector.tensor_copy(out=o_sb, in_=ps_o)
                nc.sync.dma_start(out=out_d[sc * P:(sc + 1) * P, :], in_=o_sb)

            f_state["psc"] = ps_c_pool.tile([P, 1024], F32, name="psc")
            for pr in range(NPAIR):
                matvec(3, 1, pr, pr == 0, pr == NPAIR - 1)
                pump(63, 2)
                p4bb(pr)
            vscale(3, 1)
            pump(63, len(queue))
            for sc in range(NPAIR, QC):
                p4bb(sc)

    nc.compile()
    _CACHE["nc"] = nc
    return nc


def kernel(x: np.ndarray, w_qkv: np.ndarray, w_o: np.ndarray) -> np.ndarray:
    import ml_dtypes
    from concourse.bass_utils import run_bass_kernel_spmd

    nc = _build()

    def pair_interleave(a):
        # [1024, C] -> [512, 2C]: rows 256p+128i+part -> row 128p+part,
        # col block i
        cc = a.shape[1]
        return np.ascontiguousarray(
            a.reshape(4, 2, 128, cc).transpose(0, 2, 1, 3).reshape(512, 2 * cc))

    def to_fp8(a):
        return np.clip(a, -240.0, 240.0).astype(ml_dtypes.float8_e4m3)

    xT = [np.ascontiguousarray(x[b].T).astype(ml_dtypes.bfloat16)
          for b in range(B)]
    x8 = [to_fp8(pair_interleave(np.ascontiguousarray(x[b].T)))
          for b in range(B)]
    in_maps = []
    for c in range(N_CORES):
        b, g = divmod(c, HPC)
        base = 256 * g
        q01 = w_qkv[:, base:base + 128]
        q23 = w_qkv[:, base + 128:base + 256]
        k01 = w_qkv[:, 1024 + base:1024 + base + 128]
        k23 = w_qkv[:, 1024 + base + 128:1024 + base + 256]
        v01 = w_qkv[:, 2048 + base:2048 + base + 128]
        v23 = w_qkv[:, 2048 + base + 128:2048 + base + 256]
        wqk = np.concatenate([q01, k01, q23, k23], axis=1)
        wv = np.concatenate([v01, v23], axis=1)
        wo_slice = w_o[base:base + 256, :] * (1.0 / 1024.0)
        in_maps.append({
            "xT": xT[b],
            "x8": x8[b],
            "w8": to_fp8(pair_interleave(wqk)),
            "wqkv": wv.astype(ml_dtypes.bfloat16),
            "wo": wo_slice.astype(ml_dtypes.bfloat16),
        })

    res = run_bass_kernel_spmd(nc, in_maps, list(range(N_CORES)),
                               **_CACHE.get("run_kwargs", {}))
    _CACHE["last_result"] = res

    out = np.zeros((B, S, HIDDEN), np.float32)
    for c in range(N_CORES):
        r = res.results[c]
        out[c // HPC] += (r["out"].astype(np.float32)
                          + r["out2"].astype(np.float32)
                          + r["out3"].astype(np.float32))
    return out

